# revision 1
# baseline (speedup 1.0000x reference)
"""Trainium2 Bass kernel for nn_ComplexityAttention (GQA attention block).

Computation (B=1, S=2048, HID=2048, 16 Q heads / 4 KV heads, D=128):
  q/k/v = x @ W^T + mu @ Wm^T           (fused mu-guided projections)
  per-head RMSNorm on q, k; RoPE; causal GQA attention; out @ wo^T.

Sharding: tensor-parallel over heads across 8 NeuronCores. Core c owns
Q heads {2c, 2c+1} and KV head c//2 (KV work duplicated per core pair).
Each core produces a partial output (its heads' slice of wo applied),
host sums the 8 partials.

Device-side layout strategy:
  - Host pre-transposes x/mu to [HID, S] and weights to [HID, out] so all
    matmuls contract over the partition dim with no on-device transposes
    for the projections.
  - Projections computed in [s, d] tiles (one PSUM bank holds q0|q1|k|v),
    RMSNorm+RoPE done with per-partition scalars + free-dim shifts
    (fused scalar_tensor_tensor ops), then Q/K PE-transposed to [d, s]
    for attention.
  - Scores computed transposed: S^T[kv, q] = K^T.T @ Q^T. Softmax without
    max-subtraction (scores bounded by +/-sqrt(128) after RMSNorm, exp is
    safe in fp32); denominator via ones-vector matmul; causal masking via
    4 static multiplicative masks on the diagonal tiles.
  - PV: out^T[d, q] = V[kv, d].T @ expS^T[kv, q] accumulated over kv chunks.
  - Output projection from out^T directly; partial written as [o, s] fp32.

All matmul inputs are bf16 (fp32 PSUM accumulation); statistics in fp32.
"""

import sys

for _p in ("/opt/trn_rl_repo", "/root/.axon_site/_ro/trn_rl_repo"):
    if _p not in sys.path:
        sys.path.insert(0, _p)

import numpy as np
import ml_dtypes

import concourse.bass as bass
import concourse.bacc as bacc
import concourse.mybir as mybir
import concourse.tile as tile
from concourse.bass_utils import run_bass_kernel_spmd
from concourse.masks import make_identity

# Problem constants (hardcoded per contract)
B, S, HID = 1, 2048, 2048
NUM_HEADS, NUM_KV_HEADS, HEAD_DIM = 16, 4, 128
ROPE_THETA = 10000.0
EPS = 1e-6
N_CORES = 8

P = 128
KC = HID // P            # 16 contraction chunks
SC = S // P              # 16 sequence chunks of 128
QCH = 512                # attention q-chunk (one PSUM bank)
NQC = S // QCH           # 4
NPASS = 8                # projection passes (2 s-chunks each)
SCP = SC // NPASS        # s-chunks per pass = 2
QK_SCALE = 1.0 / float(np.sqrt(HEAD_DIM))

BF16 = mybir.dt.bfloat16
F32 = mybir.dt.float32
NP_BF16 = ml_dtypes.bfloat16

_PROGRAM = {}  # repeats -> compiled Bacc program


def _build_program(repeats=1):
    """Build the per-core Bass/Tile program (identical on all 8 cores)."""
    AF = mybir.ActivationFunctionType
    OP = mybir.AluOpType

    nc = bacc.Bacc(trn_type="TRN2", debug=False)

    # ---- DRAM I/O ----
    xT = nc.dram_tensor("xT", [KC, P, S], BF16, kind="ExternalInput")
    muT = nc.dram_tensor("muT", [KC, P, S], BF16, kind="ExternalInput")
    # packed projection weights: [q0 | q1 | k | v] columns, transposed to [HID, 512]
    w_all = nc.dram_tensor("w_all", [KC, P, 512], BF16, kind="ExternalInput")
    wm_all = nc.dram_tensor("wm_all", [KC, P, 512], BF16, kind="ExternalInput")
    woT = nc.dram_tensor("woT", [2, P, HID], BF16, kind="ExternalInput")
    cosq = nc.dram_tensor("cosq", [SC, P, HEAD_DIM], F32, kind="ExternalInput")
    sinq = nc.dram_tensor("sinq", [SC, P, HEAD_DIM], F32, kind="ExternalInput")
    cosk = nc.dram_tensor("cosk", [SC, P, HEAD_DIM], F32, kind="ExternalInput")
    sink = nc.dram_tensor("sink", [SC, P, HEAD_DIM], F32, kind="ExternalInput")
    out_d = nc.dram_tensor("out", [KC, P, S], F32, kind="ExternalOutput")

    with tile.TileContext(nc) as tc:
        with (
            tc.tile_pool(name="persist", bufs=1) as persist,
            tc.tile_pool(name="stream", bufs=8) as stream,
            tc.tile_pool(name="tmp", bufs=6) as tmp,
            tc.tile_pool(name="small", bufs=6) as small,
            tc.tile_pool(name="expp", bufs=6) as expp,
            tc.tile_pool(name="ostage", bufs=6) as ostage,
            tc.tile_pool(name="ps_big", bufs=6, space="PSUM") as ps_big,
            tc.tile_pool(name="ps_scr", bufs=2, space="PSUM") as ps_scr,
        ):
            # ---- persistent SBUF tensors ----
            w_sb = persist.tile([P, KC, 512], BF16, name="w_sb")
            wm_sb = persist.tile([P, KC, 512], BF16, name="wm_sb")
            wo_sb = persist.tile([P, 2, HID], BF16, name="wo_sb")
            cq_sb = persist.tile([P, SC, HEAD_DIM], F32, name="cq_sb")
            sq_sb = persist.tile([P, SC, HEAD_DIM], F32, name="sq_sb")
            ck_sb = persist.tile([P, SC, HEAD_DIM], F32, name="ck_sb")
            sk_sb = persist.tile([P, SC, HEAD_DIM], F32, name="sk_sb")
            qt_sb = [
                persist.tile([P, S], BF16, name=f"qt{h}_sb") for h in range(2)
            ]
            kt_sb = persist.tile([P, S], BF16, name="kt_sb")
            v_sb = persist.tile([P, SC, HEAD_DIM], BF16, name="v_sb")
            attn_sb = [
                persist.tile([P, S], BF16, name=f"attn{c}_sb") for c in range(2)
            ]
            ident = persist.tile([P, P], BF16, name="ident")
            ones_sb = persist.tile([P, 1], BF16, name="ones_sb")
            eps_sb = persist.tile([P, 1], F32, name="eps_sb")
            masks = [
                persist.tile([P, P], BF16, name=f"mask{r}") for r in range(1)
            ]

            make_identity(nc, ident[:])
            nc.gpsimd.memset(ones_sb[:], 1.0)
            nc.gpsimd.memset(eps_sb[:], EPS)
            for r in range(1):
                # keep 1.0 where (q_local - kv_local) >= 0, else 0
                nc.gpsimd.memset(masks[r][:], 1.0)
                nc.gpsimd.affine_select(
                    out=masks[r][:],
                    in_=masks[r][:],
                    compare_op=mybir.AluOpType.is_ge,
                    fill=0.0,
                    base=0,
                    pattern=[[1, P]],
                    channel_multiplier=-1,
                )

            # head offsets inside the packed 512-wide projection output
            # (q0, q1, k occupy 0:128, 128:256, 256:384 and get norm+rope;
            #  v occupies 384:512)
            norm_specs = [
                (2, ck_sb, sk_sb, kt_sb),
                (0, cq_sb, sq_sb, qt_sb[0]),
                (1, cq_sb, sq_sb, qt_sb[1]),
            ]

            def attention_scores_pv(qc):
                """scores/exp/PV/den accumulation for q chunk qc; returns
                (out_ps, den_ps) per head."""
                jpq = QCH // P  # kv chunks per q chunk
                jmax = jpq * qc + (jpq - 1)
                q_sl = slice(qc * QCH, (qc + 1) * QCH)
                out_ps = [
                    ps_big.tile([P, QCH], F32, tag="big", name=f"out_ps{h}")
                    for h in range(2)
                ]
                den_ps = [
                    ps_scr.tile([1, QCH], F32, tag="scr", name=f"den_ps{h}")
                    for h in range(2)
                ]
                for j in range(jmax + 1):
                    r = j - jpq * qc
                    for h in range(2):
                        s_ps = ps_big.tile([P, QCH], F32, tag="big", name="s_ps")
                        nc.tensor.matmul(
                            s_ps[:],
                            kt_sb[:, j * P : (j + 1) * P],
                            qt_sb[h][:, q_sl],
                            start=True,
                            stop=True,
                        )
                        e = expp.tile([P, QCH], BF16, tag="e", name="e")
                        if r > 0:
                            # columns < 128*r are fully masked: zero them and
                            # exp only the live tail
                            nc.vector.memset(e[:, : P * r], 0.0)
                            nc.scalar.activation(
                                e[:, P * r :], s_ps[:, P * r :], AF.Exp,
                                scale=QK_SCALE,
                            )
                        else:
                            nc.scalar.activation(
                                e[:], s_ps[:], AF.Exp, scale=QK_SCALE
                            )
                        if r >= 0:
                            # triangular mask on the 128-wide diagonal block
                            nc.vector.tensor_mul(
                                e[:, P * r : P * (r + 1)],
                                e[:, P * r : P * (r + 1)],
                                masks[0][:],
                            )
                        nc.tensor.matmul(
                            out_ps[h][:],
                            v_sb[:, j, :],
                            e[:],
                            start=(j == 0),
                            stop=(j == jmax),
                        )
                        nc.tensor.matmul(
                            den_ps[h][:],
                            ones_sb[:],
                            e[:],
                            start=(j == 0),
                            stop=(j == jmax),
                        )
                return out_ps, den_ps

            def attention_div(qc, out_ps, den_ps):
                q_sl = slice(qc * QCH, (qc + 1) * QCH)
                for h in range(2):
                    rd = small.tile([1, QCH], F32, tag="rd", name="rd")
                    nc.vector.reciprocal(rd[:], den_ps[h][:])
                    rdb = tmp.tile([P, QCH], F32, tag="rdb", name="rdb")
                    nc.gpsimd.partition_broadcast(rdb[:], rd[:])
                    nc.vector.tensor_mul(
                        attn_sb[h][:, q_sl], out_ps[h][:], rdb[:]
                    )

            def do_wo(qc):
                """output projection for q chunk qc: out_pT[o, q] partial."""
                q_sl = slice(qc * QCH, (qc + 1) * QCH)
                for oc in range(KC):
                    o_ps = ps_big.tile([P, QCH], F32, tag="big", name="o_ps")
                    for c in range(2):
                        nc.tensor.matmul(
                            o_ps[:],
                            wo_sb[:, c, oc * P : (oc + 1) * P],
                            attn_sb[c][:, q_sl],
                            start=(c == 0),
                            stop=(c == 1),
                        )
                    ob = ostage.tile([P, QCH], F32, tag="ob", name="ob")
                    nc.vector.tensor_copy(ob[:], o_ps[:])
                    nc.scalar.dma_start(
                        out_d.ap()[oc, :, q_sl], ob[:]
                    )

            for rep in range(repeats):
                for p in range(NPASS):
                    col0 = p * SCP * P  # first s column of this pass (512 wide)
                    psums = [
                        ps_big.tile([P, 512], F32, tag="big", name=f"proj{p}_{i}")
                        for i in range(SCP)
                    ]
                    # x @ W^T contributions
                    for kc in range(KC):
                        if p == 0 and rep == 0:
                            nc.scalar.dma_start(w_sb[:, kc, :], w_all.ap()[kc])
                        xt = stream.tile([P, SCP * P], BF16, tag="xt", name="xt")
                        nc.sync.dma_start(xt[:], xT.ap()[kc, :, col0 : col0 + SCP * P])
                        for i in range(SCP):
                            nc.tensor.matmul(
                                psums[i][:],
                                xt[:, i * P : (i + 1) * P],
                                w_sb[:, kc, :],
                                start=(kc == 0),
                                stop=False,
                            )
                    # mu @ Wm^T contributions
                    for kc in range(KC):
                        if p == 0 and rep == 0:
                            nc.scalar.dma_start(wm_sb[:, kc, :], wm_all.ap()[kc])
                        mt = stream.tile([P, SCP * P], BF16, tag="mt", name="mt")
                        nc.sync.dma_start(mt[:], muT.ap()[kc, :, col0 : col0 + SCP * P])
                        for i in range(SCP):
                            nc.tensor.matmul(
                                psums[i][:],
                                mt[:, i * P : (i + 1) * P],
                                wm_sb[:, kc, :],
                                start=False,
                                stop=(kc == KC - 1),
                            )
                    if p == 0 and rep == 0:
                        for sc4 in range(0, SC, 4):
                            nc.scalar.dma_start(
                                cq_sb[:, sc4 : sc4 + 4, :],
                                cosq.ap()[sc4 : sc4 + 4].rearrange("s p d -> p s d"),
                            )
                            nc.scalar.dma_start(
                                sq_sb[:, sc4 : sc4 + 4, :],
                                sinq.ap()[sc4 : sc4 + 4].rearrange("s p d -> p s d"),
                            )
                            nc.scalar.dma_start(
                                ck_sb[:, sc4 : sc4 + 4, :],
                                cosk.ap()[sc4 : sc4 + 4].rearrange("s p d -> p s d"),
                            )
                            nc.scalar.dma_start(
                                sk_sb[:, sc4 : sc4 + 4, :],
                                sink.ap()[sc4 : sc4 + 4].rearrange("s p d -> p s d"),
                            )
                        for c in range(2):
                            nc.scalar.dma_start(wo_sb[:, c, :], woT.ap()[c])
                    # RMSNorm + RoPE + transpose to [d, s]; V copy
                    for i in range(SCP):
                        sc = p * SCP + i
                        ps = psums[i]
                        for hidx, c_sb, s_sb, dst in norm_specs:
                            off = hidx * P
                            sqv = tmp.tile([P, HEAD_DIM], F32, tag="sqv", name="sqv")
                            var = small.tile([P, 1], F32, tag="var", name="var")
                            nc.scalar.activation(
                                sqv[:], ps[:, off : off + P], AF.Square, accum_out=var[:]
                            )
                            std = small.tile([P, 1], F32, tag="std", name="std")
                            # std = sqrt(sum(q^2)/D + eps)
                            nc.scalar.activation(
                                std[:], var[:], AF.Sqrt, scale=1.0 / HEAD_DIM, bias=eps_sb[:]
                            )
                            rstd = small.tile([P, 1], F32, tag="rstd", name="rstd")
                            nc.vector.reciprocal(rstd[:], std[:])
                            t1 = tmp.tile([P, HEAD_DIM], F32, tag="t1", name="t1")
                            nc.vector.scalar_tensor_tensor(
                                t1[:],
                                ps[:, off : off + P],
                                rstd[:],
                                c_sb[:, sc, :],
                                op0=OP.mult,
                                op1=OP.mult,
                            )
                            t2 = tmp.tile([P, HEAD_DIM], F32, tag="t2", name="t2")
                            nc.vector.scalar_tensor_tensor(
                                t2[:, 0:64],
                                ps[:, off + 64 : off + P],
                                rstd[:],
                                s_sb[:, sc, 0:64],
                                op0=OP.mult,
                                op1=OP.mult,
                            )
                            nc.vector.scalar_tensor_tensor(
                                t2[:, 64:P],
                                ps[:, off : off + 64],
                                rstd[:],
                                s_sb[:, sc, 64:P],
                                op0=OP.mult,
                                op1=OP.mult,
                            )
                            qsd = tmp.tile([P, HEAD_DIM], BF16, tag="qsd", name="qsd")
                            nc.vector.tensor_add(qsd[:], t1[:], t2[:])
                            tr = ps_scr.tile([P, P], BF16, tag="scr", name="tr")
                            nc.tensor.transpose(tr[:], qsd[:], ident[:])
                            nc.vector.tensor_copy(
                                dst[:, sc * P : (sc + 1) * P], tr[:]
                            )
                        # V: plain copy (cast) into [s, d] layout
                        nc.scalar.copy(v_sb[:, sc, :], ps[:, 384:512])
                # attention + output projection, after all projections
                # (keeps ACT on one table set per phase: sqrt/square first, exp after)
                for qc in range(NQC):
                    acc = attention_scores_pv(qc)
                    if qc > 0:
                        do_wo(qc - 1)
                    attention_div(qc, *acc)
                do_wo(NQC - 1)


    nc.compile()
    return nc


def _get_program(repeats=1):
    if repeats not in _PROGRAM:
        _PROGRAM[repeats] = _build_program(repeats)
    return _PROGRAM[repeats]


def _host_prepare(inputs):
    """Shard + lay out inputs for the 8 cores."""
    hs = np.asarray(inputs["hidden_states"], dtype=np.float32).reshape(S, HID)
    mu = np.asarray(inputs["mu_prev"], dtype=np.float32).reshape(S, HID)
    wq = np.asarray(inputs["wq"], dtype=np.float32)
    wk = np.asarray(inputs["wk"], dtype=np.float32)
    wv = np.asarray(inputs["wv"], dtype=np.float32)
    wo = np.asarray(inputs["wo"], dtype=np.float32)
    wmq = np.asarray(inputs["wmq"], dtype=np.float32)
    wmk = np.asarray(inputs["wmk"], dtype=np.float32)
    wmv = np.asarray(inputs["wmv"], dtype=np.float32)
    qw = np.asarray(inputs["q_norm_w"], dtype=np.float32)
    kw = np.asarray(inputs["k_norm_w"], dtype=np.float32)

    xT = np.ascontiguousarray(hs.T).astype(NP_BF16).reshape(KC, P, S)
    muT = np.ascontiguousarray(mu.T).astype(NP_BF16).reshape(KC, P, S)

    # RoPE tables in [s, d] layout with rotate-half sign and norm weight baked in
    inv = 1.0 / (ROPE_THETA ** (np.arange(0, HEAD_DIM, 2, dtype=np.float32) / HEAD_DIM))
    ang = np.arange(S, dtype=np.float32)[:, None] * inv[None, :]  # [S, 64]
    emb = np.concatenate([ang, ang], axis=-1)  # [S, 128]
    cos_e = np.cos(emb)
    sin_e = np.sin(emb)
    sin_s = np.concatenate([-sin_e[:, :64], sin_e[:, 64:]], axis=-1)

    def tables(w):
        w_shift = np.concatenate([w[64:], w[:64]])
        cos_t = (cos_e * w[None, :]).astype(np.float32).reshape(SC, P, HEAD_DIM)
        sin_t = (sin_s * w_shift[None, :]).astype(np.float32).reshape(SC, P, HEAD_DIM)
        return np.ascontiguousarray(cos_t), np.ascontiguousarray(sin_t)

    cq, sq = tables(qw)
    ck, sk = tables(kw)

    in_maps = []
    for c in range(N_CORES):
        g = c // 2
        wq_s = wq[256 * c : 256 * (c + 1)]      # [256, HID]
        wmq_s = wmq[256 * c : 256 * (c + 1)]
        wk_s = wk[P * g : P * (g + 1)]          # [128, HID]
        wmk_s = wmk[P * g : P * (g + 1)]
        wv_s = wv[P * g : P * (g + 1)]
        wmv_s = wmv[P * g : P * (g + 1)]
        w_all = np.concatenate([wq_s.T, wk_s.T, wv_s.T], axis=1)     # [HID, 512]
        wm_all = np.concatenate([wmq_s.T, wmk_s.T, wmv_s.T], axis=1)
        woT_c = wo[:, 256 * c : 256 * (c + 1)].T                     # [256, HID]
        in_maps.append(
            {
                "xT": xT,
                "muT": muT,
                "w_all": np.ascontiguousarray(w_all).astype(NP_BF16).reshape(KC, P, 512),
                "wm_all": np.ascontiguousarray(wm_all).astype(NP_BF16).reshape(KC, P, 512),
                "woT": np.ascontiguousarray(woT_c).astype(NP_BF16).reshape(2, P, HID),
                "cosq": cq,
                "sinq": sq,
                "cosk": ck,
                "sink": sk,
            }
        )
    return in_maps


def run(inputs, trace=False):
    """Run the SPMD kernel; returns (full_output, exec_time_ns_or_None)."""
    nc = _get_program()
    in_maps = _host_prepare(inputs)
    res = run_bass_kernel_spmd(
        nc, in_maps, core_ids=list(range(N_CORES)), trace=trace
    )
    total = np.zeros((HID, S), dtype=np.float32)
    for c in range(N_CORES):
        total += res.results[c]["out"].reshape(HID, S)
    out = np.ascontiguousarray(total.T).reshape(B, S, HID).astype(np.float32)
    return out, res.exec_time_ns


def kernel(**inputs) -> np.ndarray:
    out, _ = run(inputs, trace=False)
    return out



# revision 7
# speedup vs baseline: 1.7878x; 1.7878x over previous
"""Trainium2 Bass kernel for nn_ComplexityAttention (GQA attention block).

Computation (B=1, S=2048, HID=2048, 16 Q heads / 4 KV heads, D=128):
  q/k/v = x @ W^T + mu @ Wm^T           (fused mu-guided projections)
  per-head RMSNorm on q, k; RoPE; causal GQA attention; out @ wo^T.

Sharding: tensor-parallel over heads across 8 NeuronCores. Core c owns
Q heads {2c, 2c+1} and KV head c//2 (KV work duplicated per core pair).
Each core produces a partial output (its heads' slice of wo applied),
host sums the 8 partials.

Key performance structure (vs the naive version):
  - All DMAs are batched (one per pass per tensor, one per weight tensor,
    one output store per q-chunk) to keep the single-slot HWDGE
    descriptor engine off the critical path.
  - mu-side projections run in fp8(e4m3) with DoubleRow perf mode
    (2 contraction planes per instruction at 0.5 cycles/row): the mu
    contribution is 10x smaller than the x contribution, so fp8
    quantization error there is negligible. Weights are pre-scaled by
    2^7 so fp8 wm stays in the normal range; the scale cancels in
    q/k RMSNorm and is divided out of v during the PSUM->SBUF copy.
  - Projection matmuls are issued chunk-major so PSUM banks release
    early and the norm/rope/transpose drain overlaps the next pass.
  - Attention is software-pipelined: scores for kv-chunk j issue ahead
    of PV/den for j-1 so the scalar-engine exp hides under PE work;
    the output projection (wo) for the previous q-chunk is interleaved
    into the attention j-loop to fill PE gaps.
  - Scores computed transposed: S^T[kv, q] = K^T.T @ Q^T. Softmax
    without max-subtraction (scores bounded after RMSNorm); denominator
    via ones-vector matmul; causal masking via a static multiplicative
    mask on the diagonal tiles.

All attention matmul inputs are bf16 (fp32 PSUM accumulation);
statistics in fp32.
"""

import sys

for _p in ("/opt/trn_rl_repo", "/root/.axon_site/_ro/trn_rl_repo"):
    if _p not in sys.path:
        sys.path.insert(0, _p)

import numpy as np
import ml_dtypes

import concourse.bass as bass
import concourse.bacc as bacc
import concourse.mybir as mybir
import concourse.hw_specs as _hw_specs
import concourse.tile as tile
from concourse.bass_utils import run_bass_kernel_spmd
from concourse.masks import make_identity

# Problem constants (hardcoded per contract)
B, S, HID = 1, 2048, 2048
NUM_HEADS, NUM_KV_HEADS, HEAD_DIM = 16, 4, 128
ROPE_THETA = 10000.0
EPS = 1e-6
N_CORES = 8

P = 128
KC = HID // P            # 16 contraction chunks
SC = S // P              # 16 sequence chunks of 128
QCH = 512                # attention q-chunk (one PSUM bank)
NQC = S // QCH           # 4
NPASS = 4                # projection passes (4 s-chunks each)
SCP = SC // NPASS        # s-chunks per pass = 4
QK_SCALE = 1.0 / float(np.sqrt(HEAD_DIM))
WSCALE = 128.0           # weight pre-scale so fp8 wm stays normal-range

BF16 = mybir.dt.bfloat16
F32 = mybir.dt.float32
FP8 = mybir.dt.float8e4
NP_BF16 = ml_dtypes.bfloat16
NP_FP8 = ml_dtypes.float8_e4m3fn

_PROGRAM = {}


def _pin_act_tables():
    """Restrict the activation-table advertisement so every function this
    kernel uses (Square, Ln, Exp, Copy) resolves to the single
    natural_log_exp_and_others set: one table load for the whole kernel
    instead of Sqrt<->Exp thrash when the scheduler interleaves the
    RMSNorm chain with attention exps. Set ids stay aligned with
    act_info.json, and the chosen set genuinely contains all four
    functions, so hardware behaviour is unchanged."""
    AF = mybir.ActivationFunctionType
    if getattr(bacc, "_act_tables_pinned", False):
        return
    orig = bacc.get_activation_tables
    keep = {AF.Exp, AF.Ln, AF.Square, AF.Copy, AF.Identity}

    def pinned(module_arch):
        tabs = dict(orig(module_arch))
        if "natural_log_exp_and_others" in tabs:
            for name in tabs:
                if name != "natural_log_exp_and_others":
                    tabs[name] = set(tabs[name]) - keep
        return tabs

    bacc.get_activation_tables = pinned
    bacc._act_tables_pinned = True


def _build_program():
    """Build the per-core Bass/Tile program (identical on all 8 cores)."""
    AF = mybir.ActivationFunctionType
    OP = mybir.AluOpType
    DR = mybir.MatmulPerfMode.DoubleRow
    _pin_act_tables()

    nc = bacc.Bacc(trn_type="TRN2", debug=False)

    # ---- DRAM I/O ----
    xT = nc.dram_tensor("xT", [KC, P, S], BF16, kind="ExternalInput")
    muT = nc.dram_tensor("muT", [KC, P, S], FP8, kind="ExternalInput")
    # packed projection weights: [q0 | q1 | k | v] columns, transposed to
    # [HID, 512], pre-scaled by WSCALE
    w_all = nc.dram_tensor("w_all", [KC, P, 512], BF16, kind="ExternalInput")
    wm_all = nc.dram_tensor("wm_all", [KC, P, 512], FP8, kind="ExternalInput")
    woT = nc.dram_tensor("woT", [2, P, HID], BF16, kind="ExternalInput")
    cosq = nc.dram_tensor("cosq", [P, SC, HEAD_DIM], BF16, kind="ExternalInput")
    sinq = nc.dram_tensor("sinq", [P, SC, HEAD_DIM], BF16, kind="ExternalInput")
    cosk = nc.dram_tensor("cosk", [P, SC, HEAD_DIM], BF16, kind="ExternalInput")
    sink = nc.dram_tensor("sink", [P, SC, HEAD_DIM], BF16, kind="ExternalInput")
    out_d = nc.dram_tensor("out", [KC, P, S], BF16, kind="ExternalOutput")

    with tile.TileContext(nc) as tc:
        with (
            tc.tile_pool(name="persist", bufs=1) as persist,
            tc.tile_pool(name="xpool", bufs=2) as xpool,
            tc.tile_pool(name="mpool", bufs=2) as mpool,
            tc.tile_pool(name="tmp", bufs=6) as tmp,
            tc.tile_pool(name="small", bufs=8) as small,
            tc.tile_pool(name="expp", bufs=6) as expp,
            tc.tile_pool(name="ostage", bufs=2) as ostage,
            tc.tile_pool(name="ps_big", bufs=6, space="PSUM") as ps_big,
            tc.tile_pool(name="ps_scr", bufs=2, space="PSUM") as ps_scr,
        ):
            # ---- persistent SBUF tensors ----
            w_sb = persist.tile([P, KC, 512], BF16, name="w_sb")
            wm_sb = persist.tile([P, KC, 512], FP8, name="wm_sb")
            wo_sb = persist.tile([P, 2, HID], BF16, name="wo_sb")
            cq_sb = persist.tile([P, SC, HEAD_DIM], BF16, name="cq_sb")
            sq_sb = persist.tile([P, SC, HEAD_DIM], BF16, name="sq_sb")
            ck_sb = persist.tile([P, SC, HEAD_DIM], BF16, name="ck_sb")
            sk_sb = persist.tile([P, SC, HEAD_DIM], BF16, name="sk_sb")
            qt_sb = [persist.tile([P, S], BF16, name=f"qt{h}_sb") for h in range(2)]
            kt_sb = persist.tile([P, S], BF16, name="kt_sb")
            v_sb = persist.tile([P, SC, HEAD_DIM], BF16, name="v_sb")
            attn_sb = [persist.tile([P, S], BF16, name=f"attn{c}_sb") for c in range(2)]
            ident = persist.tile([P, P], BF16, name="ident")
            ones_sb = persist.tile([P, 1], BF16, name="ones_sb")
            eps_sb = persist.tile([P, 1], F32, name="eps_sb")
            diag_mask = persist.tile([P, P], BF16, name="diag_mask")

            make_identity(nc, ident[:])
            nc.gpsimd.memset(ones_sb[:], 1.0)
            nc.gpsimd.memset(eps_sb[:], EPS * WSCALE * WSCALE)
            # keep 1.0 where (q_local - kv_local) >= 0, else 0
            nc.gpsimd.memset(diag_mask[:], 1.0)
            nc.gpsimd.affine_select(
                out=diag_mask[:],
                in_=diag_mask[:],
                compare_op=mybir.AluOpType.is_ge,
                fill=0.0,
                base=0,
                pattern=[[1, P]],
                channel_multiplier=-1,
            )

            # ---- initial batched loads ----
            # pass-0 x/w in 4-kc chunks so the first matmuls start early;
            # everything else as single whole-tensor DMAs.
            xt0 = xpool.tile([P, KC, SCP * P], BF16, tag="xt", name="xt")
            mt0 = mpool.tile([P, KC, SCP * P], FP8, tag="mt", name="mt")
            for k4 in range(0, KC, 4):
                nc.sync.dma_start(
                    xt0[:, k4 : k4 + 4, :],
                    xT.ap()[k4 : k4 + 4, :, 0 : SCP * P].rearrange("k p s -> p k s"),
                )
                nc.sync.dma_start(
                    w_sb[:, k4 : k4 + 4, :],
                    w_all.ap()[k4 : k4 + 4].rearrange("k p w -> p k w"),
                )
            nc.sync.dma_start(
                mt0[:], muT.ap()[:, :, 0 : SCP * P].rearrange("k p s -> p k s")
            )
            nc.sync.dma_start(wm_sb[:], wm_all.ap().rearrange("k p w -> p k w"))
            nc.sync.dma_start(cq_sb[:], cosq.ap())
            nc.sync.dma_start(sq_sb[:], sinq.ap())
            nc.sync.dma_start(ck_sb[:], cosk.ap())
            nc.sync.dma_start(sk_sb[:], sink.ap())

            # (head offset, cos table, sin table, [d, s] destination)
            norm_specs = [
                (2, ck_sb, sk_sb, kt_sb),
                (0, cq_sb, sq_sb, qt_sb[0]),
                (1, cq_sb, sq_sb, qt_sb[1]),
            ]

            def norm_rope_transpose(sc, ps):
                """RMSNorm + RoPE + transpose to [d, s] for q0/q1/k; V copy."""
                for hidx, c_sb, s_sb, dst in norm_specs:
                    off = hidx * P
                    sqv = tmp.tile([P, HEAD_DIM], F32, tag="sqv", name="sqv")
                    var = small.tile([P, 1], F32, tag="var", name="var")
                    nc.scalar.activation(
                        sqv[:], ps[:, off : off + P], AF.Square, accum_out=var[:]
                    )
                    # rstd = exp(-0.5*ln(sum(q^2)/D + eps)): Ln+Exp share one
                    # ACT table with the attention exp (Sqrt does not), so the
                    # scheduler can interleave norms with attention without
                    # reloading the activation-function table. The WSCALE
                    # factor cancels against the scaled psum values.
                    lv = small.tile([P, 1], F32, tag="lv", name="lv")
                    nc.scalar.activation(
                        lv[:], var[:], AF.Ln, scale=1.0 / HEAD_DIM, bias=eps_sb[:]
                    )
                    rstd = small.tile([P, 1], F32, tag="rstd", name="rstd")
                    nc.scalar.activation(rstd[:], lv[:], AF.Exp, scale=-0.5)
                    t1 = tmp.tile([P, HEAD_DIM], F32, tag="t1", name="t1")
                    nc.vector.scalar_tensor_tensor(
                        t1[:], ps[:, off : off + P], rstd[:], c_sb[:, sc, :],
                        op0=OP.mult, op1=OP.mult,
                    )
                    t2 = tmp.tile([P, HEAD_DIM], F32, tag="t2", name="t2")
                    nc.vector.scalar_tensor_tensor(
                        t2[:, 0:64], ps[:, off + 64 : off + P], rstd[:],
                        s_sb[:, sc, 0:64], op0=OP.mult, op1=OP.mult,
                    )
                    nc.vector.scalar_tensor_tensor(
                        t2[:, 64:P], ps[:, off : off + 64], rstd[:],
                        s_sb[:, sc, 64:P], op0=OP.mult, op1=OP.mult,
                    )
                    qsd = tmp.tile([P, HEAD_DIM], BF16, tag="qsd", name="qsd")
                    nc.vector.tensor_add(qsd[:], t1[:], t2[:])
                    tr = ps_scr.tile([P, P], BF16, tag="scr", name="tr")
                    nc.tensor.transpose(tr[:], qsd[:], ident[:])
                    nc.vector.tensor_copy(dst[:, sc * P : (sc + 1) * P], tr[:])
                # V: copy with 1/WSCALE to undo the weight pre-scale
                # (on DVE: an ACT-engine Copy would thrash the activation
                # function table against Square/Sqrt every pass)
                nc.vector.tensor_scalar_mul(
                    v_sb[:, sc, :], ps[:, 384:512], 1.0 / WSCALE
                )

            # ================= projection passes =================
            xts = {0: xt0}
            mts = {0: mt0}
            for p in range(NPASS):
                col0 = p * SCP * P
                if p + 1 < NPASS:
                    ncol0 = (p + 1) * SCP * P
                    xt_n = xpool.tile([P, KC, SCP * P], BF16, tag="xt", name="xt")
                    nc.sync.dma_start(
                        xt_n[:],
                        xT.ap()[:, :, ncol0 : ncol0 + SCP * P].rearrange(
                            "k p s -> p k s"
                        ),
                    )
                    mt_n = mpool.tile([P, KC, SCP * P], FP8, tag="mt", name="mt")
                    nc.sync.dma_start(
                        mt_n[:],
                        muT.ap()[:, :, ncol0 : ncol0 + SCP * P].rearrange(
                            "k p s -> p k s"
                        ),
                    )
                    xts[p + 1] = xt_n
                    mts[p + 1] = mt_n
                    if p == 1:
                        nc.sync.dma_start(wo_sb[:], woT.ap().rearrange("c p o -> p c o"))
                xt = xts.pop(p)
                mt = mts.pop(p)
                psums = [
                    ps_big.tile([P, 512], F32, tag="big", name="proj")
                    for i in range(SCP)
                ]
                if p == 0:
                    # k-major so compute starts as soon as the first 4-kc
                    # chunks of x and w arrive
                    for k4 in range(0, KC, 4):
                        for kc in range(k4, k4 + 4):
                            for i in range(SCP):
                                nc.tensor.matmul(
                                    psums[i][:],
                                    xt[:, kc, i * P : (i + 1) * P],
                                    w_sb[:, kc, :],
                                    start=(kc == 0),
                                    stop=False,
                                )
                    for i in range(SCP):
                        for kp in range(KC // 2):
                            nc.tensor.matmul(
                                psums[i][:],
                                mt[:, 2 * kp : 2 * kp + 2, i * P : (i + 1) * P],
                                wm_sb[:, 2 * kp : 2 * kp + 2, :],
                                start=False,
                                stop=(kp == KC // 2 - 1),
                                perf_mode=DR,
                            )
                        norm_rope_transpose(p * SCP + i, psums[i])
                else:
                    # i-major: each PSUM bank releases early so the norm
                    # drain overlaps the rest of the pass
                    for i in range(SCP):
                        for kc in range(KC):
                            nc.tensor.matmul(
                                psums[i][:],
                                xt[:, kc, i * P : (i + 1) * P],
                                w_sb[:, kc, :],
                                start=(kc == 0),
                                stop=False,
                            )
                        for kp in range(KC // 2):
                            nc.tensor.matmul(
                                psums[i][:],
                                mt[:, 2 * kp : 2 * kp + 2, i * P : (i + 1) * P],
                                wm_sb[:, 2 * kp : 2 * kp + 2, :],
                                start=False,
                                stop=(kp == KC // 2 - 1),
                                perf_mode=DR,
                            )
                        norm_rope_transpose(p * SCP + i, psums[i])

            # ================= attention + output projection =================
            def emit_scores(qc, j, es):
                r = j - 4 * qc  # diagonal-block index if >= 0
                q_sl = slice(qc * QCH, (qc + 1) * QCH)
                for h in range(2):
                    s_ps = ps_big.tile([P, QCH], F32, tag="big", name="s_ps")
                    nc.tensor.matmul(
                        s_ps[:],
                        kt_sb[:, j * P : (j + 1) * P],
                        qt_sb[h][:, q_sl],
                        start=True,
                        stop=True,
                    )
                    e = expp.tile([P, QCH], BF16, tag="e", name="e")
                    if r > 0:
                        # columns < 128*r are fully masked: zero them and
                        # exp only the live tail
                        nc.vector.memset(e[:, : P * r], 0.0)
                        nc.scalar.activation(
                            e[:, P * r :], s_ps[:, P * r :], AF.Exp, scale=QK_SCALE
                        )
                    else:
                        nc.scalar.activation(e[:], s_ps[:], AF.Exp, scale=QK_SCALE)
                    if r >= 0:
                        # triangular mask on the 128-wide diagonal block
                        nc.vector.tensor_mul(
                            e[:, P * r : P * (r + 1)],
                            e[:, P * r : P * (r + 1)],
                            diag_mask[:],
                        )
                    es[(j, h)] = e

            def emit_pv(qc, j, jmax, es, out_ps, den_ps):
                for h in range(2):
                    e = es.pop((j, h))
                    nc.tensor.matmul(
                        out_ps[h][:], v_sb[:, j, :], e[:],
                        start=(j == 0), stop=(j == jmax),
                    )
                    nc.tensor.matmul(
                        den_ps[h][:], ones_sb[:], e[:],
                        start=(j == 0), stop=(j == jmax),
                    )

            def emit_wo_chunk(qc_prev, oc, stage):
                q_sl = slice(qc_prev * QCH, (qc_prev + 1) * QCH)
                o_ps = ps_big.tile([P, QCH], F32, tag="big", name="o_ps")
                for c in range(2):
                    nc.tensor.matmul(
                        o_ps[:],
                        wo_sb[:, c, oc * P : (oc + 1) * P],
                        attn_sb[c][:, q_sl],
                        start=(c == 0),
                        stop=(c == 1),
                    )
                nc.vector.tensor_copy(stage[:, oc, :], o_ps[:])

            def emit_div(qc, out_ps, den_ps):
                q_sl = slice(qc * QCH, (qc + 1) * QCH)
                for h in range(2):
                    rd = small.tile([1, QCH], F32, tag="rd", name="rd")
                    nc.vector.reciprocal(rd[:], den_ps[h][:])
                    rdb = tmp.tile([P, QCH], F32, tag="rdb", name="rdb")
                    nc.gpsimd.partition_broadcast(rdb[:], rd[:])
                    nc.vector.tensor_mul(attn_sb[h][:, q_sl], out_ps[h][:], rdb[:])

            def flush_stage(qc_prev, stage):
                q_sl = slice(qc_prev * QCH, (qc_prev + 1) * QCH)
                nc.sync.dma_start(
                    out_d.ap()[:, :, q_sl].rearrange("k p s -> p k s"), stage[:]
                )

            for qc in range(NQC):
                jmax = 4 * qc + 3
                out_ps = [
                    ps_big.tile([P, QCH], F32, tag="big", name="out_ps")
                    for h in range(2)
                ]
                den_ps = [
                    ps_scr.tile([1, QCH], F32, tag="scr", name="den_ps")
                    for h in range(2)
                ]
                es = {}
                # wo chunks of the previous q-chunk interleave into this
                # j-loop (after the first pv) to fill PE gaps
                wo_todo = list(range(KC)) if qc > 0 else []
                stage = (
                    ostage.tile([P, KC, QCH], BF16, tag="st", name="st")
                    if qc > 0
                    else None
                )
                n_slots = max(jmax, 1)
                per_slot = (len(wo_todo) + n_slots - 1) // n_slots if wo_todo else 0

                emit_scores(qc, 0, es)
                for j in range(1, jmax + 1):
                    emit_scores(qc, j, es)
                    emit_pv(qc, j - 1, jmax, es, out_ps, den_ps)
                    for _ in range(per_slot):
                        if wo_todo:
                            emit_wo_chunk(qc - 1, wo_todo.pop(0), stage)
                emit_pv(qc, jmax, jmax, es, out_ps, den_ps)
                while wo_todo:
                    emit_wo_chunk(qc - 1, wo_todo.pop(0), stage)
                if qc > 0:
                    flush_stage(qc - 1, stage)
                emit_div(qc, out_ps, den_ps)

            # final q-chunk's output projection (flush in 4-oc pieces so
            # the store DMA overlaps the remaining wo matmuls)
            q_sl = slice((NQC - 1) * QCH, NQC * QCH)
            stage = ostage.tile([P, KC, QCH], BF16, tag="st", name="st")
            for oc in range(KC):
                emit_wo_chunk(NQC - 1, oc, stage)
                if oc % 4 == 3:
                    nc.sync.dma_start(
                        out_d.ap()[oc - 3 : oc + 1, :, q_sl].rearrange(
                            "k p s -> p k s"
                        ),
                        stage[:, oc - 3 : oc + 1, :],
                    )

    nc.compile()
    return nc


def _get_program(repeats=1):
    if repeats not in _PROGRAM:
        _PROGRAM[repeats] = _build_program()
    return _PROGRAM[repeats]


def _host_prepare(inputs):
    """Shard + lay out inputs for the 8 cores."""
    hs = np.asarray(inputs["hidden_states"], dtype=np.float32).reshape(S, HID)
    mu = np.asarray(inputs["mu_prev"], dtype=np.float32).reshape(S, HID)
    wq = np.asarray(inputs["wq"], dtype=np.float32)
    wk = np.asarray(inputs["wk"], dtype=np.float32)
    wv = np.asarray(inputs["wv"], dtype=np.float32)
    wo = np.asarray(inputs["wo"], dtype=np.float32)
    wmq = np.asarray(inputs["wmq"], dtype=np.float32)
    wmk = np.asarray(inputs["wmk"], dtype=np.float32)
    wmv = np.asarray(inputs["wmv"], dtype=np.float32)
    qw = np.asarray(inputs["q_norm_w"], dtype=np.float32)
    kw = np.asarray(inputs["k_norm_w"], dtype=np.float32)

    xT = np.ascontiguousarray(hs.T).astype(NP_BF16).reshape(KC, P, S)
    muT = np.ascontiguousarray(mu.T).astype(NP_FP8).reshape(KC, P, S)

    # RoPE tables in [s, d] layout with rotate-half sign and norm weight baked in
    inv = 1.0 / (ROPE_THETA ** (np.arange(0, HEAD_DIM, 2, dtype=np.float32) / HEAD_DIM))
    ang = np.arange(S, dtype=np.float32)[:, None] * inv[None, :]  # [S, 64]
    emb = np.concatenate([ang, ang], axis=-1)  # [S, 128]
    cos_e = np.cos(emb)
    sin_e = np.sin(emb)
    sin_s = np.concatenate([-sin_e[:, :64], sin_e[:, 64:]], axis=-1)

    def tables(w):
        w_shift = np.concatenate([w[64:], w[:64]])
        # [S, D] -> [SC, P, D] -> [P, SC, D] partition-major so the load is
        # contiguous per partition (4KB descriptors)
        cos_t = (cos_e * w[None, :]).reshape(SC, P, HEAD_DIM).transpose(1, 0, 2)
        sin_t = (sin_s * w_shift[None, :]).reshape(SC, P, HEAD_DIM).transpose(1, 0, 2)
        return (np.ascontiguousarray(cos_t).astype(NP_BF16),
                np.ascontiguousarray(sin_t).astype(NP_BF16))

    cq, sq = tables(qw)
    ck, sk = tables(kw)

    in_maps = []
    for c in range(N_CORES):
        g = c // 2
        wq_s = wq[256 * c : 256 * (c + 1)]      # [256, HID]
        wmq_s = wmq[256 * c : 256 * (c + 1)]
        wk_s = wk[P * g : P * (g + 1)]          # [128, HID]
        wmk_s = wmk[P * g : P * (g + 1)]
        wv_s = wv[P * g : P * (g + 1)]
        wmv_s = wmv[P * g : P * (g + 1)]
        w_all = np.concatenate([wq_s.T, wk_s.T, wv_s.T], axis=1) * WSCALE
        wm_all = np.concatenate([wmq_s.T, wmk_s.T, wmv_s.T], axis=1) * WSCALE
        woT_c = wo[:, 256 * c : 256 * (c + 1)].T                     # [256, HID]
        in_maps.append(
            {
                "xT": xT,
                "muT": muT,
                "w_all": np.ascontiguousarray(w_all).astype(NP_BF16).reshape(KC, P, 512),
                "wm_all": np.ascontiguousarray(wm_all).astype(NP_FP8).reshape(KC, P, 512),
                "woT": np.ascontiguousarray(woT_c).astype(NP_BF16).reshape(2, P, HID),
                "cosq": cq,
                "sinq": sq,
                "cosk": ck,
                "sink": sk,
            }
        )
    return in_maps


def run(inputs, trace=False):
    """Run the SPMD kernel; returns (full_output, exec_time_ns_or_None)."""
    nc = _get_program()
    in_maps = _host_prepare(inputs)
    res = run_bass_kernel_spmd(
        nc, in_maps, core_ids=list(range(N_CORES)), trace=trace
    )
    total = np.zeros((HID, S), dtype=np.float32)
    for c in range(N_CORES):
        total += res.results[c]["out"].reshape(HID, S).astype(np.float32)
    out = np.ascontiguousarray(total.T).reshape(B, S, HID).astype(np.float32)
    return out, res.exec_time_ns


def kernel(**inputs) -> np.ndarray:
    out, _ = run(inputs, trace=False)
    return out


# revision 11
# speedup vs baseline: 1.9027x; 1.0643x over previous
"""Trainium2 Bass kernel for nn_ComplexityAttention (GQA attention block).

Computation (B=1, S=2048, HID=2048, 16 Q heads / 4 KV heads, D=128):
  q/k/v = x @ W^T + mu @ Wm^T           (fused mu-guided projections)
  per-head RMSNorm on q, k; RoPE; causal GQA attention; out @ wo^T.

Sharding: tensor-parallel over heads across 8 NeuronCores. Core c owns
Q heads {2c, 2c+1} and KV head c//2 (KV work duplicated per core pair).
Each core produces a partial output (its heads' slice of wo applied),
host sums the 8 partials.

Key performance structure (vs the naive version):
  - All DMAs are batched (one per pass per tensor, one per weight tensor,
    one output store per q-chunk) to keep the single-slot HWDGE
    descriptor engine off the critical path.
  - mu-side projections run in fp8(e4m3) with DoubleRow perf mode
    (2 contraction planes per instruction at 0.5 cycles/row): the mu
    contribution is 10x smaller than the x contribution, so fp8
    quantization error there is negligible. Weights are pre-scaled by
    2^7 so fp8 wm stays in the normal range; the scale cancels in
    q/k RMSNorm and is divided out of v during the PSUM->SBUF copy.
  - Projection matmuls are issued chunk-major so PSUM banks release
    early and the norm/rope/transpose drain overlaps the next pass.
  - Attention is software-pipelined: scores for kv-chunk j issue ahead
    of PV/den for j-1 so the scalar-engine exp hides under PE work;
    the output projection (wo) for the previous q-chunk is interleaved
    into the attention j-loop to fill PE gaps.
  - Scores computed transposed: S^T[kv, q] = K^T.T @ Q^T. Softmax
    without max-subtraction (scores bounded after RMSNorm); denominator
    via ones-vector matmul; causal masking via a static multiplicative
    mask on the diagonal tiles.

All attention matmul inputs are bf16 (fp32 PSUM accumulation);
statistics in fp32.
"""

import sys

for _p in ("/opt/trn_rl_repo", "/root/.axon_site/_ro/trn_rl_repo"):
    if _p not in sys.path:
        sys.path.insert(0, _p)

import numpy as np
import ml_dtypes

import concourse.bass as bass
import concourse.bass_isa as bass_isa
import concourse.bacc as bacc
import concourse.mybir as mybir
import concourse.hw_specs as _hw_specs
import concourse.tile as tile
from concourse.bass_utils import run_bass_kernel_spmd
from concourse.masks import make_identity

# Problem constants (hardcoded per contract)
B, S, HID = 1, 2048, 2048
NUM_HEADS, NUM_KV_HEADS, HEAD_DIM = 16, 4, 128
ROPE_THETA = 10000.0
EPS = 1e-6
N_CORES = 8

P = 128
KC = HID // P            # 16 contraction chunks
SC = S // P              # 16 sequence chunks of 128
QCH = 512                # attention q-chunk (one PSUM bank)
NQC = S // QCH           # 4
NPASS = 4                # projection passes (4 s-chunks each)
SCP = SC // NPASS        # s-chunks per pass = 4
QK_SCALE = 1.0 / float(np.sqrt(HEAD_DIM))
WSCALE = 128.0           # weight pre-scale so fp8 wm stays normal-range

BF16 = mybir.dt.bfloat16
FP16 = mybir.dt.float16
F32 = mybir.dt.float32
FP8 = mybir.dt.float8e4
NP_BF16 = ml_dtypes.bfloat16
NP_FP8 = ml_dtypes.float8_e4m3fn

_PROGRAM = {}


def _pin_act_tables():
    """Restrict the activation-table advertisement so every function this
    kernel uses (Square, Ln, Exp, Copy) resolves to the single
    natural_log_exp_and_others set: one table load for the whole kernel
    instead of Sqrt<->Exp thrash when the scheduler interleaves the
    RMSNorm chain with attention exps. Set ids stay aligned with
    act_info.json, and the chosen set genuinely contains all four
    functions, so hardware behaviour is unchanged."""
    AF = mybir.ActivationFunctionType
    if getattr(bacc, "_act_tables_pinned", False):
        return
    orig = bacc.get_activation_tables
    keep = {AF.Exp, AF.Ln, AF.Square, AF.Copy, AF.Identity}

    def pinned(module_arch):
        tabs = dict(orig(module_arch))
        if "natural_log_exp_and_others" in tabs:
            for name in tabs:
                if name != "natural_log_exp_and_others":
                    tabs[name] = set(tabs[name]) - keep
        return tabs

    bacc.get_activation_tables = pinned
    bacc._act_tables_pinned = True


def _build_program():
    """Build the per-core Bass/Tile program (identical on all 8 cores)."""
    AF = mybir.ActivationFunctionType
    OP = mybir.AluOpType
    DR = mybir.MatmulPerfMode.DoubleRow
    _pin_act_tables()

    nc = bacc.Bacc(trn_type="TRN2", debug=False)

    # ---- DRAM I/O ----
    xT = nc.dram_tensor("xT", [KC, P, S], BF16, kind="ExternalInput")
    muT = nc.dram_tensor("muT", [KC, P, S], FP8, kind="ExternalInput")
    # packed projection weights: [q0 | q1 | k | v] columns, transposed to
    # [HID, 512], pre-scaled by WSCALE
    w_all = nc.dram_tensor("w_all", [KC, P, 512], BF16, kind="ExternalInput")
    wm_all = nc.dram_tensor("wm_all", [KC, P, 512], FP8, kind="ExternalInput")
    woT = nc.dram_tensor("woT", [2, P, HID], BF16, kind="ExternalInput")
    cosq = nc.dram_tensor("cosq", [P, SC, HEAD_DIM], BF16, kind="ExternalInput")
    sinq = nc.dram_tensor("sinq", [P, SC, HEAD_DIM], BF16, kind="ExternalInput")
    cosk = nc.dram_tensor("cosk", [P, SC, HEAD_DIM], BF16, kind="ExternalInput")
    sink = nc.dram_tensor("sink", [P, SC, HEAD_DIM], BF16, kind="ExternalInput")
    out_d = nc.dram_tensor("out", [KC, P, S], BF16, kind="ExternalOutput")

    with tile.TileContext(nc) as tc:
        with (
            tc.tile_pool(name="persist", bufs=1) as persist,
            tc.tile_pool(name="xpool", bufs=2) as xpool,
            tc.tile_pool(name="mpool", bufs=2) as mpool,
            tc.tile_pool(name="tmp", bufs=6) as tmp,
            tc.tile_pool(name="small", bufs=8) as small,
            tc.tile_pool(name="expp", bufs=6) as expp,
            tc.tile_pool(name="esump", bufs=2) as esump,
            tc.tile_pool(name="ostage", bufs=2) as ostage,
            tc.tile_pool(name="ps_big", bufs=6, space="PSUM") as ps_big,
            tc.tile_pool(name="ps_scr", bufs=2, space="PSUM") as ps_scr,
        ):
            # ---- persistent SBUF tensors ----
            w_sb = persist.tile([P, KC, 512], BF16, name="w_sb")
            wm_sb = persist.tile([P, KC, 512], FP8, name="wm_sb")
            wo_sb = persist.tile([P, 2, HID], BF16, name="wo_sb")
            cq_sb = persist.tile([P, SC, HEAD_DIM], BF16, name="cq_sb")
            sq_sb = persist.tile([P, SC, HEAD_DIM], BF16, name="sq_sb")
            ck_sb = persist.tile([P, SC, HEAD_DIM], BF16, name="ck_sb")
            sk_sb = persist.tile([P, SC, HEAD_DIM], BF16, name="sk_sb")
            qt_sb = [persist.tile([P, S], BF16, name=f"qt{h}_sb") for h in range(2)]
            kt_sb = persist.tile([P, S], BF16, name="kt_sb")
            v_sb = persist.tile([P, SC, HEAD_DIM], BF16, name="v_sb")
            attn_sb = [persist.tile([P, S], BF16, name=f"attn{c}_sb") for c in range(2)]
            ident = persist.tile([P, P], BF16, name="ident")
            ones_sb = persist.tile([P, 1], BF16, name="ones_sb")
            eps_sb = persist.tile([P, 1], F32, name="eps_sb")
            diag_mask = persist.tile([P, P], BF16, name="diag_mask")

            make_identity(nc, ident[:])
            nc.gpsimd.memset(ones_sb[:], 1.0)
            nc.gpsimd.memset(eps_sb[:], EPS * WSCALE * WSCALE)
            # keep 1.0 where (q_local - kv_local) >= 0, else 0
            nc.gpsimd.memset(diag_mask[:], 1.0)
            nc.gpsimd.affine_select(
                out=diag_mask[:],
                in_=diag_mask[:],
                compare_op=mybir.AluOpType.is_ge,
                fill=0.0,
                base=0,
                pattern=[[1, P]],
                channel_multiplier=-1,
            )

            # ---- initial batched loads ----
            # pass-0 x/w in 4-kc chunks so the first matmuls start early;
            # everything else as single whole-tensor DMAs.
            xt0 = xpool.tile([P, KC, SCP * P], BF16, tag="xt", name="xt")
            mt0 = mpool.tile([P, KC, SCP * P], FP8, tag="mt", name="mt")
            for k4 in range(0, KC, 4):
                nc.sync.dma_start(
                    xt0[:, k4 : k4 + 4, :],
                    xT.ap()[k4 : k4 + 4, :, 0 : SCP * P].rearrange("k p s -> p k s"),
                )
                nc.sync.dma_start(
                    w_sb[:, k4 : k4 + 4, :],
                    w_all.ap()[k4 : k4 + 4].rearrange("k p w -> p k w"),
                )
            nc.sync.dma_start(
                mt0[:], muT.ap()[:, :, 0 : SCP * P].rearrange("k p s -> p k s")
            )
            nc.sync.dma_start(wm_sb[:], wm_all.ap().rearrange("k p w -> p k w"))
            nc.sync.dma_start(cq_sb[:], cosq.ap())
            nc.sync.dma_start(sq_sb[:], sinq.ap())
            nc.sync.dma_start(ck_sb[:], cosk.ap())
            nc.sync.dma_start(sk_sb[:], sink.ap())

            # (head offset, cos table, sin table, [d, s] destination)
            norm_specs = [
                (2, ck_sb, sk_sb, kt_sb),
                (0, cq_sb, sq_sb, qt_sb[0]),
                (1, cq_sb, sq_sb, qt_sb[1]),
            ]

            def norm_rope_transpose(sc, ps):
                """RMSNorm + RoPE + transpose to [d, s] for q0/q1/k; V copy."""
                for hidx, c_sb, s_sb, dst in norm_specs:
                    off = hidx * P
                    sqv = tmp.tile([P, HEAD_DIM], F32, tag="sqv", name="sqv")
                    var = small.tile([P, 1], F32, tag="var", name="var")
                    nc.scalar.activation(
                        sqv[:], ps[:, off : off + P], AF.Square, accum_out=var[:]
                    )
                    # rstd = exp(-0.5*ln(sum(q^2)/D + eps)): Ln+Exp share one
                    # ACT table with the attention exp (Sqrt does not), so the
                    # scheduler can interleave norms with attention without
                    # reloading the activation-function table. The WSCALE
                    # factor cancels against the scaled psum values.
                    lv = small.tile([P, 1], F32, tag="lv", name="lv")
                    nc.scalar.activation(
                        lv[:], var[:], AF.Ln, scale=1.0 / HEAD_DIM, bias=eps_sb[:]
                    )
                    rstd = small.tile([P, 1], F32, tag="rstd", name="rstd")
                    nc.scalar.activation(rstd[:], lv[:], AF.Exp, scale=-0.5)
                    t1 = tmp.tile([P, HEAD_DIM], F32, tag="t1", name="t1")
                    nc.vector.scalar_tensor_tensor(
                        t1[:], ps[:, off : off + P], rstd[:], c_sb[:, sc, :],
                        op0=OP.mult, op1=OP.mult,
                    )
                    t2 = tmp.tile([P, HEAD_DIM], F32, tag="t2", name="t2")
                    nc.vector.scalar_tensor_tensor(
                        t2[:, 0:64], ps[:, off + 64 : off + P], rstd[:],
                        s_sb[:, sc, 0:64], op0=OP.mult, op1=OP.mult,
                    )
                    nc.vector.scalar_tensor_tensor(
                        t2[:, 64:P], ps[:, off : off + 64], rstd[:],
                        s_sb[:, sc, 64:P], op0=OP.mult, op1=OP.mult,
                    )
                    qsd = tmp.tile([P, HEAD_DIM], BF16, tag="qsd", name="qsd")
                    nc.vector.tensor_add(qsd[:], t1[:], t2[:])
                    tr = ps_scr.tile([P, P], BF16, tag="scr", name="tr")
                    nc.tensor.transpose(tr[:], qsd[:], ident[:])
                    nc.vector.tensor_copy(dst[:, sc * P : (sc + 1) * P], tr[:])
                # V: copy with 1/WSCALE to undo the weight pre-scale
                # (on DVE: an ACT-engine Copy would thrash the activation
                # function table against Square/Sqrt every pass)
                nc.vector.tensor_scalar_mul(
                    v_sb[:, sc, :], ps[:, 384:512], 1.0 / WSCALE
                )

            # ================= projection passes =================
            xts = {0: xt0}
            mts = {0: mt0}
            for p in range(NPASS):
                col0 = p * SCP * P
                if p + 1 < NPASS:
                    ncol0 = (p + 1) * SCP * P
                    xt_n = xpool.tile([P, KC, SCP * P], BF16, tag="xt", name="xt")
                    nc.sync.dma_start(
                        xt_n[:],
                        xT.ap()[:, :, ncol0 : ncol0 + SCP * P].rearrange(
                            "k p s -> p k s"
                        ),
                    )
                    mt_n = mpool.tile([P, KC, SCP * P], FP8, tag="mt", name="mt")
                    nc.sync.dma_start(
                        mt_n[:],
                        muT.ap()[:, :, ncol0 : ncol0 + SCP * P].rearrange(
                            "k p s -> p k s"
                        ),
                    )
                    xts[p + 1] = xt_n
                    mts[p + 1] = mt_n
                    if p == 1:
                        nc.sync.dma_start(wo_sb[:], woT.ap().rearrange("c p o -> p c o"))
                xt = xts.pop(p)
                mt = mts.pop(p)
                psums = [
                    ps_big.tile([P, 512], F32, tag="big", name="proj")
                    for i in range(SCP)
                ]
                if p == 0:
                    # k-major so compute starts as soon as the first 4-kc
                    # chunks of x and w arrive
                    for k4 in range(0, KC, 4):
                        for kc in range(k4, k4 + 4):
                            for i in range(SCP):
                                nc.tensor.matmul(
                                    psums[i][:],
                                    xt[:, kc, i * P : (i + 1) * P],
                                    w_sb[:, kc, :],
                                    start=(kc == 0),
                                    stop=False,
                                )
                    for i in range(SCP):
                        for kp in range(KC // 2):
                            nc.tensor.matmul(
                                psums[i][:],
                                mt[:, 2 * kp : 2 * kp + 2, i * P : (i + 1) * P],
                                wm_sb[:, 2 * kp : 2 * kp + 2, :],
                                start=False,
                                stop=(kp == KC // 2 - 1),
                                perf_mode=DR,
                            )
                        norm_rope_transpose(p * SCP + i, psums[i])
                else:
                    # i-major: each PSUM bank releases early so the norm
                    # drain overlaps the rest of the pass
                    for i in range(SCP):
                        for kc in range(KC):
                            nc.tensor.matmul(
                                psums[i][:],
                                xt[:, kc, i * P : (i + 1) * P],
                                w_sb[:, kc, :],
                                start=(kc == 0),
                                stop=False,
                            )
                        for kp in range(KC // 2):
                            nc.tensor.matmul(
                                psums[i][:],
                                mt[:, 2 * kp : 2 * kp + 2, i * P : (i + 1) * P],
                                wm_sb[:, 2 * kp : 2 * kp + 2, :],
                                start=False,
                                stop=(kp == KC // 2 - 1),
                                perf_mode=DR,
                            )
                        norm_rope_transpose(p * SCP + i, psums[i])

            # ================= attention + output projection =================
            def emit_scores(qc, j, es):
                r = j - 4 * qc  # diagonal-block index if >= 0
                q_sl = slice(qc * QCH, (qc + 1) * QCH)
                for h in range(2):
                    s_ps = ps_big.tile([P, QCH], F32, tag="big", name="s_ps")
                    nc.tensor.matmul(
                        s_ps[:],
                        kt_sb[:, j * P : (j + 1) * P],
                        qt_sb[h][:, q_sl],
                        start=True,
                        stop=True,
                    )
                    e = expp.tile([P, QCH], BF16, tag="e", name="e")
                    if r > 0:
                        # columns < 128*r are fully masked: zero them and
                        # exp only the live tail
                        nc.gpsimd.memset(e[:, : P * r], 0.0)
                        nc.scalar.activation(
                            e[:, P * r :], s_ps[:, P * r :], AF.Exp, scale=QK_SCALE
                        )
                    else:
                        nc.scalar.activation(e[:], s_ps[:], AF.Exp, scale=QK_SCALE)
                    if r >= 0:
                        # triangular mask on the 128-wide diagonal block
                        nc.vector.tensor_mul(
                            e[:, P * r : P * (r + 1)],
                            e[:, P * r : P * (r + 1)],
                            diag_mask[:],
                        )
                    es[(j, h)] = e

            def emit_pv(qc, j, jmax, es, out_ps, esum):
                for h in range(2):
                    e = es.pop((j, h))
                    nc.tensor.matmul(
                        out_ps[h][:], v_sb[:, j, :], e[:],
                        start=(j == 0), stop=(j == jmax),
                    )
                    # softmax denominator: accumulate exp tiles on DVE
                    # (2-byte packed SBUF operands run in the fast DVE mode)
                    # instead of burning PE cycles on ones-vector matmuls
                    if j == 0:
                        nc.vector.tensor_copy(esum[h][:], e[:])
                    else:
                        nc.vector.tensor_add(esum[h][:], esum[h][:], e[:])

            def emit_wo_chunk(qc_prev, oc, stage):
                q_sl = slice(qc_prev * QCH, (qc_prev + 1) * QCH)
                o_ps = ps_big.tile([P, QCH], F32, tag="big", name="o_ps")
                for c in range(2):
                    nc.tensor.matmul(
                        o_ps[:],
                        wo_sb[:, c, oc * P : (oc + 1) * P],
                        attn_sb[c][:, q_sl],
                        start=(c == 0),
                        stop=(c == 1),
                    )
                if oc % 2 == 0:
                    nc.vector.tensor_copy(stage[:, oc, :], o_ps[:])
                else:
                    nc.scalar.copy(stage[:, oc, :], o_ps[:])

            def emit_div(qc, out_ps, esum):
                q_sl = slice(qc * QCH, (qc + 1) * QCH)
                for h in range(2):
                    den = tmp.tile([P, QCH], F32, tag="rdb", name="den")
                    nc.gpsimd.partition_all_reduce(
                        den[:], esum[h][:], channels=P,
                        reduce_op=bass_isa.ReduceOp.add,
                    )
                    rdb = tmp.tile([P, QCH], F32, tag="rdb", name="rdb")
                    nc.vector.reciprocal(rdb[:], den[:])
                    nc.vector.tensor_mul(attn_sb[h][:, q_sl], out_ps[h][:], rdb[:])

            def flush_stage(qc_prev, stage):
                q_sl = slice(qc_prev * QCH, (qc_prev + 1) * QCH)
                nc.sync.dma_start(
                    out_d.ap()[:, :, q_sl].rearrange("k p s -> p k s"), stage[:]
                )

            for qc in range(NQC):
                jmax = 4 * qc + 3
                out_ps = [
                    ps_big.tile([P, QCH], F32, tag="big", name="out_ps")
                    for h in range(2)
                ]
                esum = [
                    esump.tile([P, QCH], FP16, tag="es", name="esum")
                    for h in range(2)
                ]
                es = {}
                # wo chunks of the previous q-chunk interleave into this
                # j-loop (after the first pv) to fill PE gaps
                wo_todo = list(range(KC)) if qc > 0 else []
                stage = (
                    ostage.tile([P, KC, QCH], BF16, tag="st", name="st")
                    if qc > 0
                    else None
                )
                n_slots = max(jmax, 1)
                per_slot = (len(wo_todo) + n_slots - 1) // n_slots if wo_todo else 0

                emit_scores(qc, 0, es)
                for j in range(1, jmax + 1):
                    emit_scores(qc, j, es)
                    emit_pv(qc, j - 1, jmax, es, out_ps, esum)
                    for _ in range(per_slot):
                        if wo_todo:
                            emit_wo_chunk(qc - 1, wo_todo.pop(0), stage)
                emit_pv(qc, jmax, jmax, es, out_ps, esum)
                emit_div(qc, out_ps, esum)
                while wo_todo:
                    emit_wo_chunk(qc - 1, wo_todo.pop(0), stage)
                if qc > 0:
                    flush_stage(qc - 1, stage)

            # final q-chunk's output projection (flush in 4-oc pieces so
            # the store DMA overlaps the remaining wo matmuls)
            q_sl = slice((NQC - 1) * QCH, NQC * QCH)
            stage = ostage.tile([P, KC, QCH], BF16, tag="st", name="st")
            for oc in range(KC):
                emit_wo_chunk(NQC - 1, oc, stage)
                if oc % 4 == 3:
                    nc.sync.dma_start(
                        out_d.ap()[oc - 3 : oc + 1, :, q_sl].rearrange(
                            "k p s -> p k s"
                        ),
                        stage[:, oc - 3 : oc + 1, :],
                    )

    nc.compile()
    return nc


def _get_program(repeats=1):
    if repeats not in _PROGRAM:
        _PROGRAM[repeats] = _build_program()
    return _PROGRAM[repeats]


def _host_prepare(inputs):
    """Shard + lay out inputs for the 8 cores."""
    hs = np.asarray(inputs["hidden_states"], dtype=np.float32).reshape(S, HID)
    mu = np.asarray(inputs["mu_prev"], dtype=np.float32).reshape(S, HID)
    wq = np.asarray(inputs["wq"], dtype=np.float32)
    wk = np.asarray(inputs["wk"], dtype=np.float32)
    wv = np.asarray(inputs["wv"], dtype=np.float32)
    wo = np.asarray(inputs["wo"], dtype=np.float32)
    wmq = np.asarray(inputs["wmq"], dtype=np.float32)
    wmk = np.asarray(inputs["wmk"], dtype=np.float32)
    wmv = np.asarray(inputs["wmv"], dtype=np.float32)
    qw = np.asarray(inputs["q_norm_w"], dtype=np.float32)
    kw = np.asarray(inputs["k_norm_w"], dtype=np.float32)

    xT = np.ascontiguousarray(hs.T).astype(NP_BF16).reshape(KC, P, S)
    muT = np.ascontiguousarray(mu.T).astype(NP_FP8).reshape(KC, P, S)

    # RoPE tables in [s, d] layout with rotate-half sign and norm weight baked in
    inv = 1.0 / (ROPE_THETA ** (np.arange(0, HEAD_DIM, 2, dtype=np.float32) / HEAD_DIM))
    ang = np.arange(S, dtype=np.float32)[:, None] * inv[None, :]  # [S, 64]
    emb = np.concatenate([ang, ang], axis=-1)  # [S, 128]
    cos_e = np.cos(emb)
    sin_e = np.sin(emb)
    sin_s = np.concatenate([-sin_e[:, :64], sin_e[:, 64:]], axis=-1)

    def tables(w):
        w_shift = np.concatenate([w[64:], w[:64]])
        # [S, D] -> [SC, P, D] -> [P, SC, D] partition-major so the load is
        # contiguous per partition (4KB descriptors)
        cos_t = (cos_e * w[None, :]).reshape(SC, P, HEAD_DIM).transpose(1, 0, 2)
        sin_t = (sin_s * w_shift[None, :]).reshape(SC, P, HEAD_DIM).transpose(1, 0, 2)
        return (np.ascontiguousarray(cos_t).astype(NP_BF16),
                np.ascontiguousarray(sin_t).astype(NP_BF16))

    cq, sq = tables(qw)
    ck, sk = tables(kw)

    in_maps = []
    for c in range(N_CORES):
        g = c // 2
        wq_s = wq[256 * c : 256 * (c + 1)]      # [256, HID]
        wmq_s = wmq[256 * c : 256 * (c + 1)]
        wk_s = wk[P * g : P * (g + 1)]          # [128, HID]
        wmk_s = wmk[P * g : P * (g + 1)]
        wv_s = wv[P * g : P * (g + 1)]
        wmv_s = wmv[P * g : P * (g + 1)]
        w_all = np.concatenate([wq_s.T, wk_s.T, wv_s.T], axis=1) * WSCALE
        wm_all = np.concatenate([wmq_s.T, wmk_s.T, wmv_s.T], axis=1) * WSCALE
        woT_c = wo[:, 256 * c : 256 * (c + 1)].T                     # [256, HID]
        in_maps.append(
            {
                "xT": xT,
                "muT": muT,
                "w_all": np.ascontiguousarray(w_all).astype(NP_BF16).reshape(KC, P, 512),
                "wm_all": np.ascontiguousarray(wm_all).astype(NP_FP8).reshape(KC, P, 512),
                "woT": np.ascontiguousarray(woT_c).astype(NP_BF16).reshape(2, P, HID),
                "cosq": cq,
                "sinq": sq,
                "cosk": ck,
                "sink": sk,
            }
        )
    return in_maps


def run(inputs, trace=False):
    """Run the SPMD kernel; returns (full_output, exec_time_ns_or_None)."""
    nc = _get_program()
    in_maps = _host_prepare(inputs)
    res = run_bass_kernel_spmd(
        nc, in_maps, core_ids=list(range(N_CORES)), trace=trace
    )
    total = np.zeros((HID, S), dtype=np.float32)
    for c in range(N_CORES):
        total += res.results[c]["out"].reshape(HID, S).astype(np.float32)
    out = np.ascontiguousarray(total.T).reshape(B, S, HID).astype(np.float32)
    return out, res.exec_time_ns


def kernel(**inputs) -> np.ndarray:
    out, _ = run(inputs, trace=False)
    return out


# revision 25
# speedup vs baseline: 2.0283x; 1.0660x over previous
"""Trainium2 Bass kernel for nn_ComplexityAttention (GQA attention block).

Computation (B=1, S=2048, HID=2048, 16 Q heads / 4 KV heads, D=128):
  q/k/v = x @ W^T + mu @ Wm^T           (fused mu-guided projections)
  per-head RMSNorm on q, k; RoPE; causal GQA attention; out @ wo^T.

Sharding: tensor-parallel over heads across 8 NeuronCores. Core c owns
Q heads {2c, 2c+1} and KV head c//2 (KV work duplicated per core pair).
Each core produces a partial output (its heads' slice of wo applied),
host sums the 8 partials.

Key performance structure (vs the naive version):
  - All DMAs are batched (one per pass per tensor, one per weight tensor,
    one output store per q-chunk) to keep the single-slot HWDGE
    descriptor engine off the critical path.
  - mu-side projections run in fp8(e4m3) with DoubleRow perf mode
    (2 contraction planes per instruction at 0.5 cycles/row): the mu
    contribution is 10x smaller than the x contribution, so fp8
    quantization error there is negligible. Weights are pre-scaled by
    2^7 so fp8 wm stays in the normal range; the scale cancels in
    q/k RMSNorm and is divided out of v during the PSUM->SBUF copy.
  - Projection matmuls are issued chunk-major so PSUM banks release
    early and the norm/rope/transpose drain overlaps the next pass.
  - Attention is software-pipelined: scores for kv-chunk j issue ahead
    of PV/den for j-1 so the scalar-engine exp hides under PE work;
    the output projection (wo) for the previous q-chunk is interleaved
    into the attention j-loop to fill PE gaps.
  - Scores computed transposed: S^T[kv, q] = K^T.T @ Q^T. Softmax
    without max-subtraction (scores bounded after RMSNorm); denominator
    via ones-vector matmul; causal masking via a static multiplicative
    mask on the diagonal tiles.

All attention matmul inputs are bf16 (fp32 PSUM accumulation);
statistics in fp32.
"""

import sys

for _p in ("/opt/trn_rl_repo", "/root/.axon_site/_ro/trn_rl_repo"):
    if _p not in sys.path:
        sys.path.insert(0, _p)

import numpy as np
import ml_dtypes

import concourse.bass as bass
import concourse.bass_isa as bass_isa
import concourse.bacc as bacc
import concourse.mybir as mybir
import concourse.hw_specs as _hw_specs
import concourse.tile as tile
from concourse.bass_utils import run_bass_kernel_spmd
from concourse.masks import make_identity

# Problem constants (hardcoded per contract)
B, S, HID = 1, 2048, 2048
NUM_HEADS, NUM_KV_HEADS, HEAD_DIM = 16, 4, 128
ROPE_THETA = 10000.0
EPS = 1e-6
N_CORES = 8

P = 128
KC = HID // P            # 16 contraction chunks
SC = S // P              # 16 sequence chunks of 128
QCH = 512                # attention q-chunk (one PSUM bank)
NQC = S // QCH           # 4
NPASS = 4                # projection passes (4 s-chunks each)
SCP = SC // NPASS        # s-chunks per pass = 4
QK_SCALE = 1.0 / float(np.sqrt(HEAD_DIM))
WSCALE = 128.0           # weight pre-scale so fp8 wm stays normal-range

BF16 = mybir.dt.bfloat16
FP16 = mybir.dt.float16
F32 = mybir.dt.float32
FP8 = mybir.dt.float8e4
NP_BF16 = ml_dtypes.bfloat16
NP_FP8 = ml_dtypes.float8_e4m3fn

_PROGRAM = {}


def _pin_act_tables():
    """Restrict the activation-table advertisement so every function this
    kernel uses (Square, Ln, Exp, Copy) resolves to the single
    natural_log_exp_and_others set: one table load for the whole kernel
    instead of Sqrt<->Exp thrash when the scheduler interleaves the
    RMSNorm chain with attention exps. Set ids stay aligned with
    act_info.json, and the chosen set genuinely contains all four
    functions, so hardware behaviour is unchanged."""
    AF = mybir.ActivationFunctionType
    if getattr(bacc, "_act_tables_pinned", False):
        return
    orig = bacc.get_activation_tables
    keep = {AF.Exp, AF.Ln, AF.Square, AF.Copy, AF.Identity}

    def pinned(module_arch):
        tabs = dict(orig(module_arch))
        if "natural_log_exp_and_others" in tabs:
            for name in tabs:
                if name != "natural_log_exp_and_others":
                    tabs[name] = set(tabs[name]) - keep
        return tabs

    bacc.get_activation_tables = pinned
    bacc._act_tables_pinned = True


def _build_program():
    """Build the per-core Bass/Tile program (identical on all 8 cores)."""
    AF = mybir.ActivationFunctionType
    OP = mybir.AluOpType
    DR = mybir.MatmulPerfMode.DoubleRow
    _pin_act_tables()

    nc = bacc.Bacc(trn_type="TRN2", debug=False)

    # ---- DRAM I/O ----
    xT = nc.dram_tensor("xT", [KC, P, S], BF16, kind="ExternalInput")
    muT = nc.dram_tensor("muT", [KC, P, S], FP8, kind="ExternalInput")
    # packed projection weights: [q0 | q1 | k | v] columns, transposed to
    # [HID, 512], pre-scaled by WSCALE
    w_all = nc.dram_tensor("w_all", [KC, P, 512], BF16, kind="ExternalInput")
    wm_all = nc.dram_tensor("wm_all", [KC, P, 512], FP8, kind="ExternalInput")
    woT = nc.dram_tensor("woT", [2, P, HID], BF16, kind="ExternalInput")
    cosq = nc.dram_tensor("cosq", [P, SC, HEAD_DIM], BF16, kind="ExternalInput")
    sinq = nc.dram_tensor("sinq", [P, SC, HEAD_DIM], BF16, kind="ExternalInput")
    cosk = nc.dram_tensor("cosk", [P, SC, HEAD_DIM], BF16, kind="ExternalInput")
    sink = nc.dram_tensor("sink", [P, SC, HEAD_DIM], BF16, kind="ExternalInput")
    out_d = nc.dram_tensor("out", [KC, P, S], BF16, kind="ExternalOutput")

    with tile.TileContext(nc) as tc:
        with (
            tc.tile_pool(name="persist", bufs=1) as persist,
            tc.tile_pool(name="xpool", bufs=3) as xpool,
            tc.tile_pool(name="mpool", bufs=2) as mpool,
            tc.tile_pool(name="tmp", bufs=6) as tmp,
            tc.tile_pool(name="small", bufs=8) as small,
            tc.tile_pool(name="expp", bufs=8) as expp,
            tc.tile_pool(name="esump", bufs=2) as esump,
            tc.tile_pool(name="ostage", bufs=2) as ostage,
            tc.tile_pool(name="ps_big", bufs=6, space="PSUM") as ps_big,
            tc.tile_pool(name="ps_scr", bufs=2, space="PSUM") as ps_scr,
        ):
            # ---- persistent SBUF tensors ----
            w_sb = persist.tile([P, KC, 512], BF16, name="w_sb")
            wm_sb = persist.tile([P, KC, 512], FP8, name="wm_sb")
            wo_sb = persist.tile([P, 2, HID], BF16, name="wo_sb")
            cq_sb = persist.tile([P, SC, HEAD_DIM], BF16, name="cq_sb")
            sq_sb = persist.tile([P, SC, HEAD_DIM], BF16, name="sq_sb")
            ck_sb = persist.tile([P, SC, HEAD_DIM], BF16, name="ck_sb")
            sk_sb = persist.tile([P, SC, HEAD_DIM], BF16, name="sk_sb")
            qt_sb = [persist.tile([P, S], BF16, name=f"qt{h}_sb") for h in range(2)]
            kt_sb = persist.tile([P, S], BF16, name="kt_sb")
            v_sb = persist.tile([P, SC, HEAD_DIM], BF16, name="v_sb")
            attn_sb = [persist.tile([P, S], BF16, name=f"attn{c}_sb") for c in range(2)]
            ident = persist.tile([P, P], BF16, name="ident")
            ones_sb = persist.tile([P, 1], BF16, name="ones_sb")
            eps_sb = persist.tile([P, 1], F32, name="eps_sb")
            diag_mask = persist.tile([P, P], BF16, name="diag_mask")

            make_identity(nc, ident[:])
            nc.gpsimd.memset(ones_sb[:], 1.0)
            nc.gpsimd.memset(eps_sb[:], EPS * WSCALE * WSCALE)
            # keep 1.0 where (q_local - kv_local) >= 0, else 0
            nc.gpsimd.memset(diag_mask[:], 1.0)
            nc.gpsimd.affine_select(
                out=diag_mask[:],
                in_=diag_mask[:],
                compare_op=mybir.AluOpType.is_ge,
                fill=0.0,
                base=0,
                pattern=[[1, P]],
                channel_multiplier=-1,
            )

            # ---- initial batched loads ----
            # pass-0 x/w in 4-kc chunks so the first matmuls start early;
            # everything else as single whole-tensor DMAs.
            xt0 = xpool.tile([P, KC, SCP * P], BF16, tag="xt", name="xt")
            mt0 = mpool.tile([P, KC, SCP * P], FP8, tag="mt", name="mt")
            for k4 in range(0, KC, 4):
                nc.sync.dma_start(
                    xt0[:, k4 : k4 + 4, :],
                    xT.ap()[k4 : k4 + 4, :, 0 : SCP * P].rearrange("k p s -> p k s"),
                )
                nc.sync.dma_start(
                    w_sb[:, k4 : k4 + 4, :],
                    w_all.ap()[k4 : k4 + 4].rearrange("k p w -> p k w"),
                )
            nc.sync.dma_start(
                mt0[:], muT.ap()[:, :, 0 : SCP * P].rearrange("k p s -> p k s")
            )
            nc.sync.dma_start(wm_sb[:], wm_all.ap().rearrange("k p w -> p k w"))

            # (head offset, cos table, sin table, [d, s] destination)
            norm_specs = [
                (2, ck_sb, sk_sb, kt_sb),
                (0, cq_sb, sq_sb, qt_sb[0]),
                (1, cq_sb, sq_sb, qt_sb[1]),
            ]

            def norm_rope_transpose(sc, ps):
                """RMSNorm + RoPE + transpose to [d, s] for q0/q1/k; V copy."""
                for hidx, c_sb, s_sb, dst in norm_specs:
                    off = hidx * P
                    sqv = tmp.tile([P, HEAD_DIM], F32, tag="sqv", name="sqv")
                    var = small.tile([P, 1], F32, tag="var", name="var")
                    nc.scalar.activation(
                        sqv[:], ps[:, off : off + P], AF.Square, accum_out=var[:]
                    )
                    # rstd = exp(-0.5*ln(sum(q^2)/D + eps)): Ln+Exp share one
                    # ACT table with the attention exp (Sqrt does not), so the
                    # scheduler can interleave norms with attention without
                    # reloading the activation-function table. The WSCALE
                    # factor cancels against the scaled psum values.
                    lv = small.tile([P, 1], F32, tag="lv", name="lv")
                    nc.scalar.activation(
                        lv[:], var[:], AF.Ln, scale=1.0 / HEAD_DIM, bias=eps_sb[:]
                    )
                    rstd = small.tile([P, 1], F32, tag="rstd", name="rstd")
                    nc.scalar.activation(rstd[:], lv[:], AF.Exp, scale=-0.5)
                    # qn = ps*rstd on ACT (per-partition scale, PSUM-legal);
                    # the rope multiplies then run as fast 16-bit DVE ops on
                    # SBUF operands instead of fp32 PSUM-reading ones
                    qn = tmp.tile([P, HEAD_DIM], FP16, tag="qn", name="qn")
                    nc.scalar.activation(
                        qn[:], ps[:, off : off + P], AF.Copy, scale=rstd[:]
                    )
                    t1 = tmp.tile([P, HEAD_DIM], FP16, tag="t1", name="t1")
                    nc.vector.tensor_mul(t1[:], qn[:], c_sb[:, sc, :])
                    t2 = tmp.tile([P, HEAD_DIM], FP16, tag="t2", name="t2")
                    nc.vector.tensor_mul(t2[:, 0:64], qn[:, 64:P], s_sb[:, sc, 0:64])
                    nc.vector.tensor_mul(t2[:, 64:P], qn[:, 0:64], s_sb[:, sc, 64:P])
                    qsd = tmp.tile([P, HEAD_DIM], BF16, tag="qsd", name="qsd")
                    nc.vector.tensor_add(qsd[:], t1[:], t2[:])
                    tr = ps_scr.tile([P, P], BF16, tag="scr", name="tr")
                    nc.tensor.transpose(tr[:], qsd[:], ident[:])
                    nc.vector.tensor_copy(dst[:, sc * P : (sc + 1) * P], tr[:])
                # V: copy with 1/WSCALE to undo the weight pre-scale
                # (on DVE: an ACT-engine Copy would thrash the activation
                # function table against Square/Sqrt every pass)
                nc.vector.tensor_scalar_mul(
                    v_sb[:, sc, :], ps[:, 384:512], 1.0 / WSCALE
                )

            # ================= projection passes =================
            xts = {0: xt0}
            mts = {0: mt0}
            for p in range(NPASS):
                col0 = p * SCP * P
                if p + 1 < NPASS:
                    ncol0 = (p + 1) * SCP * P
                    xt_n = xpool.tile([P, KC, SCP * P], BF16, tag="xt", name="xt")
                    nc.sync.dma_start(
                        xt_n[:],
                        xT.ap()[:, :, ncol0 : ncol0 + SCP * P].rearrange(
                            "k p s -> p k s"
                        ),
                    )
                    mt_n = mpool.tile([P, KC, SCP * P], FP8, tag="mt", name="mt")
                    nc.sync.dma_start(
                        mt_n[:],
                        muT.ap()[:, :, ncol0 : ncol0 + SCP * P].rearrange(
                            "k p s -> p k s"
                        ),
                    )
                    xts[p + 1] = xt_n
                    mts[p + 1] = mt_n
                    if p == 0:
                        nc.sync.dma_start(cq_sb[:], cosq.ap())
                        nc.sync.dma_start(sq_sb[:], sinq.ap())
                        nc.sync.dma_start(ck_sb[:], cosk.ap())
                        nc.sync.dma_start(sk_sb[:], sink.ap())
                    if p == 1:
                        nc.sync.dma_start(wo_sb[:], woT.ap().rearrange("c p o -> p c o"))
                xt = xts.pop(p)
                mt = mts.pop(p)
                psums = [
                    ps_big.tile([P, 512], F32, tag="big", name="proj")
                    for i in range(SCP)
                ]
                if p == 0:
                    # k-major so compute starts as soon as the first 4-kc
                    # chunks of x and w arrive
                    for k4 in range(0, KC, 4):
                        for kc in range(k4, k4 + 4):
                            for i in range(SCP):
                                nc.tensor.matmul(
                                    psums[i][:],
                                    xt[:, kc, i * P : (i + 1) * P],
                                    w_sb[:, kc, :],
                                    start=(kc == 0),
                                    stop=False,
                                )
                    for i in range(SCP):
                        for kp in range(KC // 2):
                            nc.tensor.matmul(
                                psums[i][:],
                                mt[:, 2 * kp : 2 * kp + 2, i * P : (i + 1) * P],
                                wm_sb[:, 2 * kp : 2 * kp + 2, :],
                                start=False,
                                stop=(kp == KC // 2 - 1),
                                perf_mode=DR,
                            )
                        norm_rope_transpose(p * SCP + i, psums[i])
                else:
                    # i-major: each PSUM bank releases early so the norm
                    # drain overlaps the rest of the pass
                    for i in range(SCP):
                        for kc in range(KC):
                            nc.tensor.matmul(
                                psums[i][:],
                                xt[:, kc, i * P : (i + 1) * P],
                                w_sb[:, kc, :],
                                start=(kc == 0),
                                stop=False,
                            )
                        for kp in range(KC // 2):
                            nc.tensor.matmul(
                                psums[i][:],
                                mt[:, 2 * kp : 2 * kp + 2, i * P : (i + 1) * P],
                                wm_sb[:, 2 * kp : 2 * kp + 2, :],
                                start=False,
                                stop=(kp == KC // 2 - 1),
                                perf_mode=DR,
                            )
                        norm_rope_transpose(p * SCP + i, psums[i])

            # ================= attention + output projection =================
            def emit_scores(qc, j, es):
                r = j - 4 * qc  # diagonal-block index if >= 0
                q_sl = slice(qc * QCH, (qc + 1) * QCH)
                for h in range(2):
                    s_ps = ps_big.tile([P, QCH], F32, tag="big", name="s_ps")
                    nc.tensor.matmul(
                        s_ps[:],
                        kt_sb[:, j * P : (j + 1) * P],
                        qt_sb[h][:, q_sl],
                        start=True,
                        stop=True,
                    )
                    e = expp.tile([P, QCH], BF16, tag="e", name="e")
                    if r > 0:
                        # columns < 128*r are fully masked: zero them and
                        # exp only the live tail
                        nc.gpsimd.memset(e[:, : P * r], 0.0)
                        nc.scalar.activation(
                            e[:, P * r :], s_ps[:, P * r :], AF.Exp, scale=QK_SCALE
                        )
                    else:
                        nc.scalar.activation(e[:], s_ps[:], AF.Exp, scale=QK_SCALE)
                    if r >= 0:
                        # triangular mask on the 128-wide diagonal block
                        nc.gpsimd.tensor_mul(
                            e[:, P * r : P * (r + 1)],
                            e[:, P * r : P * (r + 1)],
                            diag_mask[:],
                        )
                    es[(j, h)] = e

            def emit_pv(qc, j, jmax, es, out_ps, esum):
                for h in range(2):
                    e = es.pop((j, h))
                    nc.tensor.matmul(
                        out_ps[h][:], v_sb[:, j, :], e[:],
                        start=(j == 0), stop=(j == jmax),
                    )
                    # softmax denominator: accumulate exp tiles on DVE
                    # (2-byte packed SBUF operands run in the fast DVE mode)
                    # instead of burning PE cycles on ones-vector matmuls
                    if j == 0:
                        nc.vector.tensor_copy(esum[h][:], e[:])
                    else:
                        nc.vector.tensor_add(esum[h][:], esum[h][:], e[:])

            def emit_wo_chunk(qc_prev, oc, stage):
                q_sl = slice(qc_prev * QCH, (qc_prev + 1) * QCH)
                o_ps = ps_big.tile([P, QCH], F32, tag="big", name="o_ps")
                for c in range(2):
                    nc.tensor.matmul(
                        o_ps[:],
                        wo_sb[:, c, oc * P : (oc + 1) * P],
                        attn_sb[c][:, q_sl],
                        start=(c == 0),
                        stop=(c == 1),
                    )
                if oc % 3 != 2:
                    nc.vector.tensor_copy(stage[:, oc, :], o_ps[:])
                else:
                    nc.scalar.copy(stage[:, oc, :], o_ps[:])

            def emit_div(qc, out_ps, esum):
                q_sl = slice(qc * QCH, (qc + 1) * QCH)
                for h in range(2):
                    den = tmp.tile([P, QCH], F32, tag="rdb", name="den")
                    nc.gpsimd.partition_all_reduce(
                        den[:], esum[h][:], channels=P,
                        reduce_op=bass_isa.ReduceOp.add,
                    )
                    rdb = tmp.tile([P, QCH], F32, tag="rdb", name="rdb")
                    nc.vector.reciprocal(rdb[:], den[:])
                    nc.vector.tensor_mul(attn_sb[h][:, q_sl], out_ps[h][:], rdb[:])

            def flush_stage(qc_prev, stage):
                q_sl = slice(qc_prev * QCH, (qc_prev + 1) * QCH)
                nc.sync.dma_start(
                    out_d.ap()[:, :, q_sl].rearrange("k p s -> p k s"), stage[:]
                )

            for qc in range(NQC):
                jmax = 4 * qc + 3
                out_ps = [
                    ps_big.tile([P, QCH], F32, tag="big", name="out_ps")
                    for h in range(2)
                ]
                esum = [
                    esump.tile([P, QCH], FP16, tag="es", name="esum")
                    for h in range(2)
                ]
                es = {}
                # wo chunks of the previous q-chunk interleave into this
                # j-loop (after the first pv) to fill PE gaps
                wo_todo = list(range(KC)) if qc > 0 else []
                stage = (
                    ostage.tile([P, KC, QCH], BF16, tag="st", name="st")
                    if qc > 0
                    else None
                )
                n_slots = max(jmax, 1)
                per_slot = (len(wo_todo) + n_slots - 1) // n_slots if wo_todo else 0

                emit_scores(qc, 0, es)
                for j in range(1, jmax + 1):
                    emit_scores(qc, j, es)
                    emit_pv(qc, j - 1, jmax, es, out_ps, esum)
                    for _ in range(per_slot):
                        if wo_todo:
                            emit_wo_chunk(qc - 1, wo_todo.pop(0), stage)
                emit_pv(qc, jmax, jmax, es, out_ps, esum)
                emit_div(qc, out_ps, esum)
                while wo_todo:
                    emit_wo_chunk(qc - 1, wo_todo.pop(0), stage)
                if qc > 0:
                    flush_stage(qc - 1, stage)

            # final q-chunk's output projection (flush in 4-oc pieces so
            # the store DMA overlaps the remaining wo matmuls)
            q_sl = slice((NQC - 1) * QCH, NQC * QCH)
            stage = ostage.tile([P, KC, QCH], BF16, tag="st", name="st")
            for oc in range(KC):
                emit_wo_chunk(NQC - 1, oc, stage)
                if oc % 2 == 1:
                    nc.sync.dma_start(
                        out_d.ap()[oc - 1 : oc + 1, :, q_sl].rearrange(
                            "k p s -> p k s"
                        ),
                        stage[:, oc - 1 : oc + 1, :],
                    )

    nc.compile()
    return nc


def _get_program(repeats=1):
    if repeats not in _PROGRAM:
        _PROGRAM[repeats] = _build_program()
    return _PROGRAM[repeats]


def _host_prepare(inputs):
    """Shard + lay out inputs for the 8 cores."""
    hs = np.asarray(inputs["hidden_states"], dtype=np.float32).reshape(S, HID)
    mu = np.asarray(inputs["mu_prev"], dtype=np.float32).reshape(S, HID)
    wq = np.asarray(inputs["wq"], dtype=np.float32)
    wk = np.asarray(inputs["wk"], dtype=np.float32)
    wv = np.asarray(inputs["wv"], dtype=np.float32)
    wo = np.asarray(inputs["wo"], dtype=np.float32)
    wmq = np.asarray(inputs["wmq"], dtype=np.float32)
    wmk = np.asarray(inputs["wmk"], dtype=np.float32)
    wmv = np.asarray(inputs["wmv"], dtype=np.float32)
    qw = np.asarray(inputs["q_norm_w"], dtype=np.float32)
    kw = np.asarray(inputs["k_norm_w"], dtype=np.float32)

    xT = np.ascontiguousarray(hs.T).astype(NP_BF16).reshape(KC, P, S)
    muT = np.ascontiguousarray(mu.T).astype(NP_FP8).reshape(KC, P, S)

    # RoPE tables in [s, d] layout with rotate-half sign and norm weight baked in
    inv = 1.0 / (ROPE_THETA ** (np.arange(0, HEAD_DIM, 2, dtype=np.float32) / HEAD_DIM))
    ang = np.arange(S, dtype=np.float32)[:, None] * inv[None, :]  # [S, 64]
    emb = np.concatenate([ang, ang], axis=-1)  # [S, 128]
    cos_e = np.cos(emb)
    sin_e = np.sin(emb)
    sin_s = np.concatenate([-sin_e[:, :64], sin_e[:, 64:]], axis=-1)

    def tables(w):
        w_shift = np.concatenate([w[64:], w[:64]])
        # [S, D] -> [SC, P, D] -> [P, SC, D] partition-major so the load is
        # contiguous per partition (4KB descriptors)
        cos_t = (cos_e * w[None, :]).reshape(SC, P, HEAD_DIM).transpose(1, 0, 2)
        sin_t = (sin_s * w_shift[None, :]).reshape(SC, P, HEAD_DIM).transpose(1, 0, 2)
        return (np.ascontiguousarray(cos_t).astype(NP_BF16),
                np.ascontiguousarray(sin_t).astype(NP_BF16))

    cq, sq = tables(qw)
    ck, sk = tables(kw)

    in_maps = []
    for c in range(N_CORES):
        g = c // 2
        wq_s = wq[256 * c : 256 * (c + 1)]      # [256, HID]
        wmq_s = wmq[256 * c : 256 * (c + 1)]
        wk_s = wk[P * g : P * (g + 1)]          # [128, HID]
        wmk_s = wmk[P * g : P * (g + 1)]
        wv_s = wv[P * g : P * (g + 1)]
        wmv_s = wmv[P * g : P * (g + 1)]
        w_all = np.concatenate([wq_s.T, wk_s.T, wv_s.T], axis=1) * WSCALE
        wm_all = np.concatenate([wmq_s.T, wmk_s.T, wmv_s.T], axis=1) * WSCALE
        woT_c = wo[:, 256 * c : 256 * (c + 1)].T                     # [256, HID]
        in_maps.append(
            {
                "xT": xT,
                "muT": muT,
                "w_all": np.ascontiguousarray(w_all).astype(NP_BF16).reshape(KC, P, 512),
                "wm_all": np.ascontiguousarray(wm_all).astype(NP_FP8).reshape(KC, P, 512),
                "woT": np.ascontiguousarray(woT_c).astype(NP_BF16).reshape(2, P, HID),
                "cosq": cq,
                "sinq": sq,
                "cosk": ck,
                "sink": sk,
            }
        )
    return in_maps


def run(inputs, trace=False):
    """Run the SPMD kernel; returns (full_output, exec_time_ns_or_None)."""
    nc = _get_program()
    in_maps = _host_prepare(inputs)
    res = run_bass_kernel_spmd(
        nc, in_maps, core_ids=list(range(N_CORES)), trace=trace
    )
    total = np.zeros((HID, S), dtype=np.float32)
    for c in range(N_CORES):
        total += res.results[c]["out"].reshape(HID, S).astype(np.float32)
    out = np.ascontiguousarray(total.T).reshape(B, S, HID).astype(np.float32)
    return out, res.exec_time_ns


def kernel(**inputs) -> np.ndarray:
    out, _ = run(inputs, trace=False)
    return out


# revision 29
# speedup vs baseline: 2.0479x; 1.0097x over previous
"""Trainium2 Bass kernel for nn_ComplexityAttention (GQA attention block).

Computation (B=1, S=2048, HID=2048, 16 Q heads / 4 KV heads, D=128):
  q/k/v = x @ W^T + mu @ Wm^T           (fused mu-guided projections)
  per-head RMSNorm on q, k; RoPE; causal GQA attention; out @ wo^T.

Sharding: tensor-parallel over heads across 8 NeuronCores. Core c owns
Q heads {2c, 2c+1} and KV head c//2 (KV work duplicated per core pair).
Each core produces a partial output (its heads' slice of wo applied),
host sums the 8 partials.

Key performance structure (vs the naive version):
  - All DMAs are batched (one per pass per tensor, one per weight tensor,
    one output store per q-chunk) to keep the single-slot HWDGE
    descriptor engine off the critical path.
  - mu-side projections run in fp8(e4m3) with DoubleRow perf mode
    (2 contraction planes per instruction at 0.5 cycles/row): the mu
    contribution is 10x smaller than the x contribution, so fp8
    quantization error there is negligible. Weights are pre-scaled by
    2^7 so fp8 wm stays in the normal range; the scale cancels in
    q/k RMSNorm and is divided out of v during the PSUM->SBUF copy.
  - Projection matmuls are issued chunk-major so PSUM banks release
    early and the norm/rope/transpose drain overlaps the next pass.
  - Attention is software-pipelined: scores for kv-chunk j issue ahead
    of PV/den for j-1 so the scalar-engine exp hides under PE work;
    the output projection (wo) for the previous q-chunk is interleaved
    into the attention j-loop to fill PE gaps.
  - Scores computed transposed: S^T[kv, q] = K^T.T @ Q^T. Softmax
    without max-subtraction (scores bounded after RMSNorm); denominator
    via ones-vector matmul; causal masking via a static multiplicative
    mask on the diagonal tiles.

All attention matmul inputs are bf16 (fp32 PSUM accumulation);
statistics in fp32.
"""

import sys

for _p in ("/opt/trn_rl_repo", "/root/.axon_site/_ro/trn_rl_repo"):
    if _p not in sys.path:
        sys.path.insert(0, _p)

import numpy as np
import ml_dtypes

import concourse.bass as bass
import concourse.bass_isa as bass_isa
import concourse.bacc as bacc
import concourse.mybir as mybir
import concourse.hw_specs as _hw_specs
import concourse.tile as tile
from concourse.bass_utils import run_bass_kernel_spmd
from concourse.masks import make_identity

# Problem constants (hardcoded per contract)
B, S, HID = 1, 2048, 2048
NUM_HEADS, NUM_KV_HEADS, HEAD_DIM = 16, 4, 128
ROPE_THETA = 10000.0
EPS = 1e-6
N_CORES = 8

P = 128
KC = HID // P            # 16 contraction chunks
SC = S // P              # 16 sequence chunks of 128
QCH = 512                # attention q-chunk (one PSUM bank)
NQC = S // QCH           # 4
NPASS = 4                # projection passes (4 s-chunks each)
SCP = SC // NPASS        # s-chunks per pass = 4
QK_SCALE = 1.0 / float(np.sqrt(HEAD_DIM))
WSCALE = 128.0           # weight pre-scale so fp8 wm stays normal-range

BF16 = mybir.dt.bfloat16
FP16 = mybir.dt.float16
F32 = mybir.dt.float32
FP8 = mybir.dt.float8e4
NP_BF16 = ml_dtypes.bfloat16
NP_FP8 = ml_dtypes.float8_e4m3fn

_PROGRAM = {}


def _pin_act_tables():
    """Restrict the activation-table advertisement so every function this
    kernel uses (Square, Ln, Exp, Copy) resolves to the single
    natural_log_exp_and_others set: one table load for the whole kernel
    instead of Sqrt<->Exp thrash when the scheduler interleaves the
    RMSNorm chain with attention exps. Set ids stay aligned with
    act_info.json, and the chosen set genuinely contains all four
    functions, so hardware behaviour is unchanged."""
    AF = mybir.ActivationFunctionType
    if getattr(bacc, "_act_tables_pinned", False):
        return
    orig = bacc.get_activation_tables
    keep = {AF.Exp, AF.Ln, AF.Square, AF.Copy, AF.Identity}

    def pinned(module_arch):
        tabs = dict(orig(module_arch))
        if "natural_log_exp_and_others" in tabs:
            for name in tabs:
                if name != "natural_log_exp_and_others":
                    tabs[name] = set(tabs[name]) - keep
        return tabs

    bacc.get_activation_tables = pinned
    bacc._act_tables_pinned = True


def _build_program():
    """Build the per-core Bass/Tile program (identical on all 8 cores)."""
    AF = mybir.ActivationFunctionType
    OP = mybir.AluOpType
    DR = mybir.MatmulPerfMode.DoubleRow
    _pin_act_tables()

    nc = bacc.Bacc(trn_type="TRN2", debug=False)

    # ---- DRAM I/O ----
    xT = nc.dram_tensor("xT", [KC, P, S], BF16, kind="ExternalInput")
    muT = nc.dram_tensor("muT", [KC, P, S], FP8, kind="ExternalInput")
    # packed projection weights: [q0 | q1 | k | v] columns, transposed to
    # [HID, 512], pre-scaled by WSCALE
    w_all = nc.dram_tensor("w_all", [KC, P, 512], BF16, kind="ExternalInput")
    wm_all = nc.dram_tensor("wm_all", [KC, P, 512], FP8, kind="ExternalInput")
    woT = nc.dram_tensor("woT", [2, P, HID], BF16, kind="ExternalInput")
    cosq = nc.dram_tensor("cosq", [P, SC, HEAD_DIM], BF16, kind="ExternalInput")
    sinq = nc.dram_tensor("sinq", [P, SC, HEAD_DIM], BF16, kind="ExternalInput")
    cosk = nc.dram_tensor("cosk", [P, SC, HEAD_DIM], BF16, kind="ExternalInput")
    sink = nc.dram_tensor("sink", [P, SC, HEAD_DIM], BF16, kind="ExternalInput")
    out_d = nc.dram_tensor("out", [KC, P, S], BF16, kind="ExternalOutput")

    with tile.TileContext(nc) as tc:
        with (
            tc.tile_pool(name="persist", bufs=1) as persist,
            tc.tile_pool(name="xpool", bufs=3) as xpool,
            tc.tile_pool(name="mpool", bufs=2) as mpool,
            tc.tile_pool(name="tmp", bufs=6) as tmp,
            tc.tile_pool(name="small", bufs=8) as small,
            tc.tile_pool(name="expp", bufs=8) as expp,
            tc.tile_pool(name="esump", bufs=2) as esump,
            tc.tile_pool(name="ostage", bufs=2) as ostage,
            tc.tile_pool(name="ps_big", bufs=6, space="PSUM") as ps_big,
            tc.tile_pool(name="ps_scr", bufs=2, space="PSUM") as ps_scr,
        ):
            # ---- persistent SBUF tensors ----
            w_sb = persist.tile([P, KC, 512], BF16, name="w_sb")
            wm_sb = persist.tile([P, KC, 512], FP8, name="wm_sb")
            wo_sb = persist.tile([P, 2, HID], BF16, name="wo_sb")
            cq_sb = persist.tile([P, SC, HEAD_DIM], BF16, name="cq_sb")
            sq_sb = persist.tile([P, SC, HEAD_DIM], BF16, name="sq_sb")
            ck_sb = persist.tile([P, SC, HEAD_DIM], BF16, name="ck_sb")
            sk_sb = persist.tile([P, SC, HEAD_DIM], BF16, name="sk_sb")
            qt_sb = [persist.tile([P, S], BF16, name=f"qt{h}_sb") for h in range(2)]
            kt_sb = persist.tile([P, S], BF16, name="kt_sb")
            v_sb = persist.tile([P, SC, HEAD_DIM], BF16, name="v_sb")
            attn_sb = [persist.tile([P, S], BF16, name=f"attn{c}_sb") for c in range(2)]
            ident = persist.tile([P, P], BF16, name="ident")
            ones_sb = persist.tile([P, 1], BF16, name="ones_sb")
            eps_sb = persist.tile([P, 1], F32, name="eps_sb")
            diag_mask = persist.tile([P, P], BF16, name="diag_mask")

            make_identity(nc, ident[:])
            nc.gpsimd.memset(ones_sb[:], 1.0)
            nc.gpsimd.memset(eps_sb[:], EPS * WSCALE * WSCALE)
            # keep 1.0 where (q_local - kv_local) >= 0, else 0
            nc.gpsimd.memset(diag_mask[:], 1.0)
            nc.gpsimd.affine_select(
                out=diag_mask[:],
                in_=diag_mask[:],
                compare_op=mybir.AluOpType.is_ge,
                fill=0.0,
                base=0,
                pattern=[[1, P]],
                channel_multiplier=-1,
            )

            # ---- initial batched loads ----
            # pass-0 x/w in 4-kc chunks so the first matmuls start early;
            # everything else as single whole-tensor DMAs.
            xt0 = xpool.tile([P, KC, SCP * P], BF16, tag="xt", name="xt")
            mt0 = mpool.tile([P, KC, SCP * P], FP8, tag="mt", name="mt")
            chunks = [(0, 2), (2, 2), (4, 4), (8, 4), (12, 4)]
            for k0, kn in chunks:
                nc.sync.dma_start(
                    xt0[:, k0 : k0 + kn, :],
                    xT.ap()[k0 : k0 + kn, :, 0 : SCP * P].rearrange("k p s -> p k s"),
                )
                nc.sync.dma_start(
                    w_sb[:, k0 : k0 + kn, :],
                    w_all.ap()[k0 : k0 + kn].rearrange("k p w -> p k w"),
                )
            nc.sync.dma_start(
                mt0[:], muT.ap()[:, :, 0 : SCP * P].rearrange("k p s -> p k s")
            )
            nc.sync.dma_start(wm_sb[:], wm_all.ap().rearrange("k p w -> p k w"))

            # (head offset, cos table, sin table, [d, s] destination)
            norm_specs = [
                (2, ck_sb, sk_sb, kt_sb),
                (0, cq_sb, sq_sb, qt_sb[0]),
                (1, cq_sb, sq_sb, qt_sb[1]),
            ]

            def norm_rope_transpose(sc, ps):
                """RMSNorm + RoPE + transpose to [d, s] for q0/q1/k; V copy."""
                for hidx, c_sb, s_sb, dst in norm_specs:
                    off = hidx * P
                    sqv = tmp.tile([P, HEAD_DIM], F32, tag="sqv", name="sqv")
                    var = small.tile([P, 1], F32, tag="var", name="var")
                    nc.scalar.activation(
                        sqv[:], ps[:, off : off + P], AF.Square, accum_out=var[:]
                    )
                    # rstd = exp(-0.5*ln(sum(q^2)/D + eps)): Ln+Exp share one
                    # ACT table with the attention exp (Sqrt does not), so the
                    # scheduler can interleave norms with attention without
                    # reloading the activation-function table. The WSCALE
                    # factor cancels against the scaled psum values.
                    lv = small.tile([P, 1], F32, tag="lv", name="lv")
                    nc.scalar.activation(
                        lv[:], var[:], AF.Ln, scale=1.0 / HEAD_DIM, bias=eps_sb[:]
                    )
                    rstd = small.tile([P, 1], F32, tag="rstd", name="rstd")
                    nc.scalar.activation(rstd[:], lv[:], AF.Exp, scale=-0.5)
                    # qn = ps*rstd on ACT (per-partition scale, PSUM-legal);
                    # the rope multiplies then run as fast 16-bit DVE ops on
                    # SBUF operands instead of fp32 PSUM-reading ones
                    qn = tmp.tile([P, HEAD_DIM], FP16, tag="qn", name="qn")
                    nc.scalar.activation(
                        qn[:], ps[:, off : off + P], AF.Copy, scale=rstd[:]
                    )
                    t1 = tmp.tile([P, HEAD_DIM], FP16, tag="t1", name="t1")
                    nc.vector.tensor_mul(t1[:], qn[:], c_sb[:, sc, :])
                    t2 = tmp.tile([P, HEAD_DIM], FP16, tag="t2", name="t2")
                    nc.vector.tensor_mul(t2[:, 0:64], qn[:, 64:P], s_sb[:, sc, 0:64])
                    nc.vector.tensor_mul(t2[:, 64:P], qn[:, 0:64], s_sb[:, sc, 64:P])
                    qsd = tmp.tile([P, HEAD_DIM], BF16, tag="qsd", name="qsd")
                    nc.vector.tensor_add(qsd[:], t1[:], t2[:])
                    tr = ps_scr.tile([P, P], BF16, tag="scr", name="tr")
                    nc.tensor.transpose(tr[:], qsd[:], ident[:])
                    nc.vector.tensor_copy(dst[:, sc * P : (sc + 1) * P], tr[:])
                # V: copy with 1/WSCALE to undo the weight pre-scale
                # (on DVE: an ACT-engine Copy would thrash the activation
                # function table against Square/Sqrt every pass)
                nc.vector.tensor_scalar_mul(
                    v_sb[:, sc, :], ps[:, 384:512], 1.0 / WSCALE
                )

            # ================= projection passes =================
            xts = {0: xt0}
            mts = {0: mt0}
            for p in range(NPASS):
                col0 = p * SCP * P
                if p + 1 < NPASS:
                    ncol0 = (p + 1) * SCP * P
                    xt_n = xpool.tile([P, KC, SCP * P], BF16, tag="xt", name="xt")
                    nc.sync.dma_start(
                        xt_n[:],
                        xT.ap()[:, :, ncol0 : ncol0 + SCP * P].rearrange(
                            "k p s -> p k s"
                        ),
                    )
                    mt_n = mpool.tile([P, KC, SCP * P], FP8, tag="mt", name="mt")
                    nc.sync.dma_start(
                        mt_n[:],
                        muT.ap()[:, :, ncol0 : ncol0 + SCP * P].rearrange(
                            "k p s -> p k s"
                        ),
                    )
                    xts[p + 1] = xt_n
                    mts[p + 1] = mt_n
                    if p == 0:
                        nc.sync.dma_start(cq_sb[:], cosq.ap())
                        nc.sync.dma_start(sq_sb[:], sinq.ap())
                        nc.sync.dma_start(ck_sb[:], cosk.ap())
                        nc.sync.dma_start(sk_sb[:], sink.ap())
                    if p == 1:
                        nc.sync.dma_start(wo_sb[:], woT.ap().rearrange("c p o -> p c o"))
                xt = xts.pop(p)
                mt = mts.pop(p)
                psums = [
                    ps_big.tile([P, 512], F32, tag="big", name="proj")
                    for i in range(SCP)
                ]
                if p == 0:
                    # k-major so compute starts as soon as the first 4-kc
                    # chunks of x and w arrive
                    for k4 in range(0, KC, 4):
                        for kc in range(k4, k4 + 4):
                            for i in range(SCP):
                                nc.tensor.matmul(
                                    psums[i][:],
                                    xt[:, kc, i * P : (i + 1) * P],
                                    w_sb[:, kc, :],
                                    start=(kc == 0),
                                    stop=False,
                                )
                    for i in range(SCP):
                        for kp in range(KC // 2):
                            nc.tensor.matmul(
                                psums[i][:],
                                mt[:, 2 * kp : 2 * kp + 2, i * P : (i + 1) * P],
                                wm_sb[:, 2 * kp : 2 * kp + 2, :],
                                start=False,
                                stop=(kp == KC // 2 - 1),
                                perf_mode=DR,
                            )
                        norm_rope_transpose(p * SCP + i, psums[i])
                else:
                    # i-major: each PSUM bank releases early so the norm
                    # drain overlaps the rest of the pass
                    for i in range(SCP):
                        for kc in range(KC):
                            nc.tensor.matmul(
                                psums[i][:],
                                xt[:, kc, i * P : (i + 1) * P],
                                w_sb[:, kc, :],
                                start=(kc == 0),
                                stop=False,
                            )
                        for kp in range(KC // 2):
                            nc.tensor.matmul(
                                psums[i][:],
                                mt[:, 2 * kp : 2 * kp + 2, i * P : (i + 1) * P],
                                wm_sb[:, 2 * kp : 2 * kp + 2, :],
                                start=False,
                                stop=(kp == KC // 2 - 1),
                                perf_mode=DR,
                            )
                        norm_rope_transpose(p * SCP + i, psums[i])

            # ================= attention + output projection =================
            def emit_scores(qc, j, es):
                r = j - 4 * qc  # diagonal-block index if >= 0
                q_sl = slice(qc * QCH, (qc + 1) * QCH)
                for h in range(2):
                    s_ps = ps_big.tile([P, QCH], F32, tag="big", name="s_ps")
                    nc.tensor.matmul(
                        s_ps[:],
                        kt_sb[:, j * P : (j + 1) * P],
                        qt_sb[h][:, q_sl],
                        start=True,
                        stop=True,
                    )
                    e = expp.tile([P, QCH], BF16, tag="e", name="e")
                    if r > 0:
                        # columns < 128*r are fully masked: zero them and
                        # exp only the live tail
                        nc.gpsimd.memset(e[:, : P * r], 0.0)
                        nc.scalar.activation(
                            e[:, P * r :], s_ps[:, P * r :], AF.Exp, scale=QK_SCALE
                        )
                    else:
                        nc.scalar.activation(e[:], s_ps[:], AF.Exp, scale=QK_SCALE)
                    if r >= 0:
                        # triangular mask on the 128-wide diagonal block
                        nc.gpsimd.tensor_mul(
                            e[:, P * r : P * (r + 1)],
                            e[:, P * r : P * (r + 1)],
                            diag_mask[:],
                        )
                    es[(j, h)] = e

            def emit_pv(qc, j, jmax, es, out_ps, esum):
                for h in range(2):
                    e = es.pop((j, h))
                    nc.tensor.matmul(
                        out_ps[h][:], v_sb[:, j, :], e[:],
                        start=(j == 0), stop=(j == jmax),
                    )
                    # softmax denominator: accumulate exp tiles on DVE
                    # (2-byte packed SBUF operands run in the fast DVE mode)
                    # instead of burning PE cycles on ones-vector matmuls
                    if j == 0:
                        nc.vector.tensor_copy(esum[h][:], e[:])
                    else:
                        nc.vector.tensor_add(esum[h][:], esum[h][:], e[:])

            def emit_wo_chunk(qc_prev, oc, stage, act_heavy=False):
                q_sl = slice(qc_prev * QCH, (qc_prev + 1) * QCH)
                o_ps = ps_big.tile([P, QCH], F32, tag="big", name="o_ps")
                for c in range(2):
                    nc.tensor.matmul(
                        o_ps[:],
                        wo_sb[:, c, oc * P : (oc + 1) * P],
                        attn_sb[c][:, q_sl],
                        start=(c == 0),
                        stop=(c == 1),
                    )
                # copy-engine split: ACT-heavy in the tail where no exps run
                dve_turn = (oc % 3 != 2) if not act_heavy else (oc % 3 == 0)
                if dve_turn:
                    nc.vector.tensor_copy(stage[:, oc, :], o_ps[:])
                else:
                    nc.scalar.copy(stage[:, oc, :], o_ps[:])

            def emit_div(qc, out_ps, esum):
                q_sl = slice(qc * QCH, (qc + 1) * QCH)
                for h in range(2):
                    den = tmp.tile([P, QCH], F32, tag="rdb", name="den")
                    nc.gpsimd.partition_all_reduce(
                        den[:], esum[h][:], channels=P,
                        reduce_op=bass_isa.ReduceOp.add,
                    )
                    rdb = tmp.tile([P, QCH], F32, tag="rdb", name="rdb")
                    nc.vector.reciprocal(rdb[:], den[:])
                    nc.vector.tensor_mul(attn_sb[h][:, q_sl], out_ps[h][:], rdb[:])

            def flush_stage(qc_prev, stage):
                q_sl = slice(qc_prev * QCH, (qc_prev + 1) * QCH)
                nc.sync.dma_start(
                    out_d.ap()[:, :, q_sl].rearrange("k p s -> p k s"), stage[:]
                )

            for qc in range(NQC):
                jmax = 4 * qc + 3
                out_ps = [
                    ps_big.tile([P, QCH], F32, tag="big", name="out_ps")
                    for h in range(2)
                ]
                esum = [
                    esump.tile([P, QCH], FP16, tag="es", name="esum")
                    for h in range(2)
                ]
                es = {}
                # wo chunks of the previous q-chunk interleave into this
                # j-loop (after the first pv) to fill PE gaps
                wo_todo = list(range(KC)) if qc > 0 else []
                stage = (
                    ostage.tile([P, KC, QCH], BF16, tag="st", name="st")
                    if qc > 0
                    else None
                )
                holdback = 0
                n_slots = max(jmax, 1)
                budget = max(len(wo_todo) - holdback, 0)
                per_slot = (budget + n_slots - 1) // n_slots if budget else 0

                emit_scores(qc, 0, es)
                for j in range(1, jmax + 1):
                    emit_scores(qc, j, es)
                    emit_pv(qc, j - 1, jmax, es, out_ps, esum)
                    for _ in range(per_slot):
                        if wo_todo and len(wo_todo) > holdback:
                            emit_wo_chunk(qc - 1, wo_todo.pop(0), stage)
                emit_pv(qc, jmax, jmax, es, out_ps, esum)
                emit_div(qc, out_ps, esum)
                while wo_todo:
                    emit_wo_chunk(qc - 1, wo_todo.pop(0), stage)
                if qc > 0:
                    flush_stage(qc - 1, stage)

            # final q-chunk's output projection (flush in 4-oc pieces so
            # the store DMA overlaps the remaining wo matmuls)
            q_sl = slice((NQC - 1) * QCH, NQC * QCH)
            stage = ostage.tile([P, KC, QCH], BF16, tag="st", name="st")
            for oc in range(KC):
                emit_wo_chunk(NQC - 1, oc, stage, act_heavy=True)
                if oc % 2 == 1:
                    nc.sync.dma_start(
                        out_d.ap()[oc - 1 : oc + 1, :, q_sl].rearrange(
                            "k p s -> p k s"
                        ),
                        stage[:, oc - 1 : oc + 1, :],
                    )

    nc.compile()
    return nc


def _get_program(repeats=1):
    if repeats not in _PROGRAM:
        _PROGRAM[repeats] = _build_program()
    return _PROGRAM[repeats]


def _host_prepare(inputs):
    """Shard + lay out inputs for the 8 cores."""
    hs = np.asarray(inputs["hidden_states"], dtype=np.float32).reshape(S, HID)
    mu = np.asarray(inputs["mu_prev"], dtype=np.float32).reshape(S, HID)
    wq = np.asarray(inputs["wq"], dtype=np.float32)
    wk = np.asarray(inputs["wk"], dtype=np.float32)
    wv = np.asarray(inputs["wv"], dtype=np.float32)
    wo = np.asarray(inputs["wo"], dtype=np.float32)
    wmq = np.asarray(inputs["wmq"], dtype=np.float32)
    wmk = np.asarray(inputs["wmk"], dtype=np.float32)
    wmv = np.asarray(inputs["wmv"], dtype=np.float32)
    qw = np.asarray(inputs["q_norm_w"], dtype=np.float32)
    kw = np.asarray(inputs["k_norm_w"], dtype=np.float32)

    xT = np.ascontiguousarray(hs.T).astype(NP_BF16).reshape(KC, P, S)
    muT = np.ascontiguousarray(mu.T).astype(NP_FP8).reshape(KC, P, S)

    # RoPE tables in [s, d] layout with rotate-half sign and norm weight baked in
    inv = 1.0 / (ROPE_THETA ** (np.arange(0, HEAD_DIM, 2, dtype=np.float32) / HEAD_DIM))
    ang = np.arange(S, dtype=np.float32)[:, None] * inv[None, :]  # [S, 64]
    emb = np.concatenate([ang, ang], axis=-1)  # [S, 128]
    cos_e = np.cos(emb)
    sin_e = np.sin(emb)
    sin_s = np.concatenate([-sin_e[:, :64], sin_e[:, 64:]], axis=-1)

    def tables(w):
        w_shift = np.concatenate([w[64:], w[:64]])
        # [S, D] -> [SC, P, D] -> [P, SC, D] partition-major so the load is
        # contiguous per partition (4KB descriptors)
        cos_t = (cos_e * w[None, :]).reshape(SC, P, HEAD_DIM).transpose(1, 0, 2)
        sin_t = (sin_s * w_shift[None, :]).reshape(SC, P, HEAD_DIM).transpose(1, 0, 2)
        return (np.ascontiguousarray(cos_t).astype(NP_BF16),
                np.ascontiguousarray(sin_t).astype(NP_BF16))

    cq, sq = tables(qw)
    ck, sk = tables(kw)

    in_maps = []
    for c in range(N_CORES):
        g = c // 2
        wq_s = wq[256 * c : 256 * (c + 1)]      # [256, HID]
        wmq_s = wmq[256 * c : 256 * (c + 1)]
        wk_s = wk[P * g : P * (g + 1)]          # [128, HID]
        wmk_s = wmk[P * g : P * (g + 1)]
        wv_s = wv[P * g : P * (g + 1)]
        wmv_s = wmv[P * g : P * (g + 1)]
        w_all = np.concatenate([wq_s.T, wk_s.T, wv_s.T], axis=1) * WSCALE
        wm_all = np.concatenate([wmq_s.T, wmk_s.T, wmv_s.T], axis=1) * WSCALE
        woT_c = wo[:, 256 * c : 256 * (c + 1)].T                     # [256, HID]
        in_maps.append(
            {
                "xT": xT,
                "muT": muT,
                "w_all": np.ascontiguousarray(w_all).astype(NP_BF16).reshape(KC, P, 512),
                "wm_all": np.ascontiguousarray(wm_all).astype(NP_FP8).reshape(KC, P, 512),
                "woT": np.ascontiguousarray(woT_c).astype(NP_BF16).reshape(2, P, HID),
                "cosq": cq,
                "sinq": sq,
                "cosk": ck,
                "sink": sk,
            }
        )
    return in_maps


def run(inputs, trace=False):
    """Run the SPMD kernel; returns (full_output, exec_time_ns_or_None)."""
    nc = _get_program()
    in_maps = _host_prepare(inputs)
    res = run_bass_kernel_spmd(
        nc, in_maps, core_ids=list(range(N_CORES)), trace=trace
    )
    total = np.zeros((HID, S), dtype=np.float32)
    for c in range(N_CORES):
        total += res.results[c]["out"].reshape(HID, S).astype(np.float32)
    out = np.ascontiguousarray(total.T).reshape(B, S, HID).astype(np.float32)
    return out, res.exec_time_ns


def kernel(**inputs) -> np.ndarray:
    out, _ = run(inputs, trace=False)
    return out
